# revision 1
# baseline (speedup 1.0000x reference)
"""STBlock (temporal attn -> spatial attn -> ChebConv + residual, relu) on 8 trn2 cores.

Sharding: data-parallel over batch B=8, one batch element per core.
Host prep: densify Chebyshev Laplacian to L (N,N), pre-transpose Vs/L, build
block-diag projection weights so Cheb+residual projections become one PSUM
accumulation group per output tile.

Per-core dataflow:
  XN[8]   (128n, 768=(f,t)) fp32 natural x tiles         <- contiguous DMA
  score_t (24,24) = sum_f sum_ntile XNf.T @ XNf           (256 small MMs, one PSUM group)
  E_att   = softmax(Ve @ sigmoid(score_t) + be)           (tiny); E4 = I4 (x) E_att, bf16
  YF4[8]  (96=(f4,u), 1024n) bf16 via 256 PE transposes of XN f-col-blocks
  TT[6]   (128=(f,t)d, 1024n) bf16 = x_TA^T: per-fgroup MM lhsT=E4 rhs=YF4
  TN[8]   (128n, 768) bf16 = x_TA natural via 48 PE transposes of TT
  SG[8]   (128, 1024) bf16 = sigmoid(score_s), score_s = TT.T@TT (bf16 MMs)
  S_att   = softmax(Vs @ SG + bs) per n-chunk (fp32 softmax); transposed -> SAT[8] bf16
  SN[8]   fp32 = x_SA = SAT.T @ TN (bf16 MMs)
  P1 = L@SN, P2 = 2*L@P1 - SN   (fp32 MMs, lhsT = LT blocks)
  per (n-chunk, t4-block): PE-transpose SN/P1/P2/XN2 col-blocks (t-major col AP)
    -> 4 accumulated fp32 MMs with block-diag W (cheb k 0..2 + residual) -> +bias, relu
  OUT rotating (128, 1536=(g,t)) -> dram (1024, 64, 24)

SBUF tag reuse (lifetimes disjoint): xn->sat->xn2, yf4->p2, tt->sn(0..5), tn->p1, sg->lt.
"""
import numpy as np

B, N, F, T, G = 8, 1024, 32, 24, 64
D = F * T            # 768
NCH = N // 128       # 8 n-chunks
DCH = D // 128       # 6 d-tiles
GT = G * T           # 1536

_compiled = {}


def _build():
    if "nc" in _compiled:
        return _compiled["nc"]
    import concourse.mybir as mybir
    import concourse.bacc as bacc
    from concourse import tile

    FP = mybir.dt.float32
    BF = mybir.dt.bfloat16
    AF = mybir.ActivationFunctionType
    OP = mybir.AluOpType
    AX = mybir.AxisListType

    nc = bacc.Bacc("TRN2", target_bir_lowering=False, debug=False)

    x_d = nc.dram_tensor("x", (N, D), FP, kind="ExternalInput").ap()
    ident_d = nc.dram_tensor("ident", (128, 128), FP, kind="ExternalInput").ap()
    vet_d = nc.dram_tensor("vet", (T, T), FP, kind="ExternalInput").ap()
    be_d = nc.dram_tensor("be", (T, T), FP, kind="ExternalInput").ap()
    vst_d = nc.dram_tensor("vst", (N, N), BF, kind="ExternalInput").ap()
    bs_d = nc.dram_tensor("bs", (N, N), FP, kind="ExternalInput").ap()
    lt_d = nc.dram_tensor("lt", (N, N), BF, kind="ExternalInput").ap()
    wbd_d = nc.dram_tensor("wbd", (4, 128, 256), BF, kind="ExternalInput").ap()
    biasf_d = nc.dram_tensor("biasf", (128, 256), FP, kind="ExternalInput").ap()
    out_d = nc.dram_tensor("out", (N, GT), FP, kind="ExternalOutput").ap()

    with tile.TileContext(nc) as tc:
        with (
            tc.tile_pool(name="persist", bufs=1) as pp,
            tc.tile_pool(name="stream", bufs=1) as sp,
            tc.tile_pool(name="psum", bufs=2, space="PSUM") as ps,
            tc.tile_pool(name="psum1", bufs=1, space="PSUM") as ps1,
        ):
            # ---- constants ----
            ident = pp.tile([128, 128], FP, tag="ident")
            nc.sync.dma_start(ident[:], ident_d[:])
            identb = pp.tile([128, 128], BF, tag="identb")
            nc.vector.tensor_copy(identb[:], ident[:])
            vet = pp.tile([T, T], FP, tag="vet")
            nc.sync.dma_start(vet[:], vet_d[:])
            be = pp.tile([T, T], FP, tag="be")
            nc.sync.dma_start(be[:], be_d[:])
            wbd = [pp.tile([128, 256], BF, name=f"wbd{k}", tag=f"wbd{k}") for k in range(4)]
            for k in range(4):
                nc.sync.dma_start(wbd[k][:], wbd_d[k])
            biasf = pp.tile([128, 256], FP, tag="biasf")
            nc.sync.dma_start(biasf[:], biasf_d[:])

            # ---- stage 0: natural x tiles (slot group A: xn -> sat -> xn2) ----
            XN = []
            for i in range(NCH):
                t = pp.tile([128, D], FP, name=f"xnA{i}", tag=f"A{i}")
                nc.sync.dma_start(t[:], x_d[i * 128:(i + 1) * 128, :])
                XN.append(t)

            # ---- stage 1: score_t ----
            ps_t = ps1.tile([T, T], FP, tag="ps_t")
            n_mm = NCH * F
            idx = 0
            for i in range(NCH):
                for f in range(F):
                    sl = XN[i][:, f * T:(f + 1) * T]
                    nc.tensor.matmul(ps_t[:], sl, sl,
                                     start=(idx == 0), stop=(idx == n_mm - 1))
                    idx += 1
            sig_t = pp.tile([T, T], FP, tag="sig_t")
            nc.scalar.activation(sig_t[:], ps_t[:], AF.Sigmoid)

            # ---- stage 2: E_att ----
            ps_e = ps1.tile([T, T], FP, tag="ps_e")
            nc.tensor.matmul(ps_e[:], vet[:], sig_t[:], start=True, stop=True)
            epre = pp.tile([T, T], FP, tag="epre")
            nc.vector.tensor_tensor(epre[:], ps_e[:], be[:], op=OP.add)
            negmax = pp.tile([T, 1], FP, tag="negmax")
            nc.vector.reduce_max(negmax[:], epre[:], axis=AX.X, negate=True)
            eexp = pp.tile([T, T], FP, tag="eexp")
            esum = pp.tile([T, 1], FP, tag="esum")
            nc.scalar.activation(eexp[:], epre[:], AF.Exp,
                                 bias=negmax[:], accum_out=esum[:])
            einv = pp.tile([T, 1], FP, tag="einv")
            nc.vector.reciprocal(einv[:], esum[:])
            eatt = pp.tile([T, T], FP, tag="eatt")
            nc.vector.tensor_scalar_mul(eatt[:], eexp[:], einv[:])
            # E4 = blockdiag(E_att x4) bf16
            e4 = pp.tile([128, 96], BF, tag="e4")
            nc.gpsimd.memset(e4[:], 0.0)
            for j in range(4):
                nc.vector.tensor_copy(e4[32 * j:32 * j + 24, 24 * j:24 * j + 24], eatt[:])

            # ---- stage 3: YF4 groups (96=(f4,u), 1024) bf16 (slot group B: yf4 -> p2) ----
            YF4 = [pp.tile([128, N], BF, name=f"yfB{g}", tag=f"B{g}") for g in range(NCH)]
            for g in range(NCH):
                nc.gpsimd.memset(YF4[g][:], 0.0)
            for i in range(NCH):
                for f in range(F):
                    pt = ps.tile([T, 128], FP, tag="ps_tr")
                    nc.tensor.transpose(pt[:], XN[i][:, f * T:(f + 1) * T], ident[:])
                    dst = YF4[f // 4][32 * (f % 4):32 * (f % 4) + 24, i * 128:(i + 1) * 128]
                    if f % 2 == 0:
                        nc.vector.tensor_copy(dst, pt[:])
                    else:
                        nc.scalar.activation(dst, pt[:], AF.Copy)

            # ---- stage 4: TT bf16 (slot group C: tt -> sn[0:6]) ----
            TT = [pp.tile([128, N], BF, name=f"ttC{p}", tag=f"C{p}") for p in range(DCH)]

            def copy_rows(dst_tiles, g0, src, rows, width):
                # copy src (rows, width) into global partition rows [g0, g0+rows);
                # 32-row pieces: nonzero-offset partition APs must stay in one quadrant
                a = g0
                while a < g0 + rows:
                    q = a // 128
                    seg = min(g0 + rows - a, 128 - (a % 128), 32)
                    s0 = a - g0
                    nc.vector.tensor_copy(dst_tiles[q][a % 128:a % 128 + seg, :width],
                                          src[s0:s0 + seg, :width])
                    a += seg

            for g in range(NCH):
                pt = ps.tile([96, N], FP, tag="ps_big")
                for half in range(2):
                    nc.tensor.matmul(pt[:, half * 512:(half + 1) * 512],
                                     e4[:],
                                     YF4[g][:, half * 512:(half + 1) * 512],
                                     start=True, stop=True)
                copy_rows(TT, 96 * g, pt[:], 96, N)

            # ---- stage 5: TN bf16 natural x_TA (slot group D: tn -> p1) ----
            TN = [pp.tile([128, D], BF, name=f"tnD{i}", tag=f"D{i}") for i in range(NCH)]
            for p in range(DCH):
                for i in range(NCH):
                    pt = ps.tile([128, 128], BF, tag="ps_tr")
                    nc.tensor.transpose(pt[:], TT[p][:, i * 128:(i + 1) * 128], identb[:])
                    dst = TN[i][:, p * 128:(p + 1) * 128]
                    if (p * NCH + i) % 2 == 0:
                        nc.vector.tensor_copy(dst, pt[:])
                    else:
                        nc.scalar.activation(dst, pt[:], AF.Copy)

            # ---- stage 6: SG bf16 (slot group E: sg -> lt) ----
            SG = [pp.tile([128, N], BF, name=f"sgE{i}", tag=f"E{i}") for i in range(NCH)]
            for i in range(NCH):
                pt = ps.tile([128, N], FP, tag="ps_big")
                for half in range(2):
                    for p in range(DCH):
                        nc.tensor.matmul(
                            pt[:, half * 512:(half + 1) * 512],
                            TT[p][:, i * 128:(i + 1) * 128],
                            TT[p][:, half * 512:(half + 1) * 512],
                            start=(p == 0), stop=(p == DCH - 1))
                nc.scalar.activation(SG[i][:], pt[:], AF.Sigmoid)

            # ---- stage 7: S_att softmax + transpose -> SAT bf16 (reuses A slots) ----
            VST = [pp.tile([128, N], BF, name=f"vst{m}", tag=f"vst{m}") for m in range(NCH)]
            for m in range(NCH):
                nc.sync.dma_start(VST[m][:], vst_d[m * 128:(m + 1) * 128, :])
            SAT = [pp.tile([128, N], BF, name=f"satA{m}", tag=f"A{m}") for m in range(NCH)]
            for i in range(NCH):
                pt = ps.tile([128, N], FP, tag="ps_big")
                for half in range(2):
                    for m in range(NCH):
                        nc.tensor.matmul(
                            pt[:, half * 512:(half + 1) * 512],
                            VST[m][:, i * 128:(i + 1) * 128],
                            SG[m][:, half * 512:(half + 1) * 512],
                            start=(m == 0), stop=(m == NCH - 1))
                bsb = sp.tile([128, N], FP, tag="bsb", bufs=2)
                nc.sync.dma_start(bsb[:], bs_d[i * 128:(i + 1) * 128, :])
                spre = sp.tile([128, N], FP, tag="spre")
                nc.vector.tensor_tensor(spre[:], pt[:], bsb[:], op=OP.add)
                nmax = sp.tile([128, 1], FP, tag="nmax")
                nc.vector.reduce_max(nmax[:], spre[:], axis=AX.X, negate=True)
                sexp = sp.tile([128, N], FP, tag="sexp")
                ssum = sp.tile([128, 1], FP, tag="ssum")
                nc.scalar.activation(sexp[:], spre[:], AF.Exp,
                                     bias=nmax[:], accum_out=ssum[:])
                sinv = sp.tile([128, 1], FP, tag="sinv")
                nc.vector.reciprocal(sinv[:], ssum[:])
                sa = sp.tile([128, N], FP, tag="sa")
                nc.vector.tensor_scalar_mul(sa[:], sexp[:], sinv[:])
                for p in range(NCH):
                    pt2 = ps.tile([128, 128], FP, tag="ps_tr")
                    nc.tensor.transpose(pt2[:], sa[:, p * 128:(p + 1) * 128], ident[:])
                    dst = SAT[p][:, i * 128:(i + 1) * 128]
                    if (i + p) % 2 == 0:
                        nc.vector.tensor_copy(dst, pt2[:])
                    else:
                        nc.scalar.activation(dst, pt2[:], AF.Copy)

            # ---- stage 8: SN fp32 = x_SA (sn[0:6] on C slots, sn6/7 fresh) ----
            SN = []
            for i in range(NCH):
                if i < DCH:
                    t = pp.tile([128, D], BF, name=f"snC{i}", tag=f"C{i}")
                else:
                    t = pp.tile([128, D], BF, name=f"sn{i}", tag=f"sn{i}")
                SN.append(t)
            for i in range(NCH):
                pt = ps.tile([128, D], FP, tag="ps_big")
                for c0, cw in ((0, 512), (512, 256)):
                    for m in range(NCH):
                        nc.tensor.matmul(
                            pt[:, c0:c0 + cw],
                            SAT[m][:, i * 128:(i + 1) * 128],
                            TN[m][:, c0:c0 + cw],
                            start=(m == 0), stop=(m == NCH - 1))
                nc.vector.tensor_copy(SN[i][:], pt[:])

            # ---- stage 9: P1 = L@SN (D slots); P2 = 2 L@P1 - SN (B slots) ----
            LT = [pp.tile([128, N], BF, name=f"ltE{m}", tag=f"E{m}") for m in range(NCH)]
            for m in range(NCH):
                nc.sync.dma_start(LT[m][:], lt_d[m * 128:(m + 1) * 128, :])
            P1 = [pp.tile([128, D], BF, name=f"p1D{i}", tag=f"D{i}") for i in range(NCH)]
            P2 = [pp.tile([128, D], BF, name=f"p2B{i}", tag=f"B{i}") for i in range(NCH)]
            for i in range(NCH):
                pt = ps.tile([128, D], FP, tag="ps_big")
                for c0, cw in ((0, 512), (512, 256)):
                    for m in range(NCH):
                        nc.tensor.matmul(
                            pt[:, c0:c0 + cw],
                            LT[m][:, i * 128:(i + 1) * 128],
                            SN[m][:, c0:c0 + cw],
                            start=(m == 0), stop=(m == NCH - 1))
                nc.vector.tensor_copy(P1[i][:], pt[:])
            for i in range(NCH):
                pt = ps.tile([128, D], FP, tag="ps_big")
                for c0, cw in ((0, 512), (512, 256)):
                    for m in range(NCH):
                        nc.tensor.matmul(
                            pt[:, c0:c0 + cw],
                            LT[m][:, i * 128:(i + 1) * 128],
                            P1[m][:, c0:c0 + cw],
                            start=(m == 0), stop=(m == NCH - 1))
                dbl = sp.tile([128, D], BF, tag="dbl")
                nc.scalar.activation(dbl[:], pt[:], AF.Copy, scale=2.0)
                nc.vector.tensor_tensor(P2[i][:], dbl[:], SN[i][:], op=OP.subtract)

            # ---- stage 10: reload x (A slots), projections + residual + relu ----
            XN2 = [pp.tile([128, D], FP, name=f"xn2A{i}", tag=f"A{i}") for i in range(NCH)]
            for i in range(NCH):
                nc.sync.dma_start(XN2[i][:], x_d[i * 128:(i + 1) * 128, :])

            for i in range(NCH):
                ob = sp.tile([128, GT], FP, tag="outbuf", bufs=2)
                srcs = (SN[i], P1[i], P2[i], XN2[i])
                # permute columns f-major -> t-major once per (chunk, tensor)
                perm = []
                for k in range(4):
                    sc = sp.tile([128, D], BF, name=f"perm{k}", tag=f"perm{k}", bufs=1)
                    s_ap = srcs[k][:].rearrange("q (f t) -> q t f", f=F, t=T)
                    d_ap = sc[:].rearrange("q (t f) -> q t f", t=T, f=F)
                    if k % 2 == 0:
                        nc.vector.tensor_copy(d_ap, s_ap)
                    else:
                        nc.scalar.activation(d_ap, s_ap, AF.Copy)
                    perm.append(sc)
                for p in range(DCH):
                    ptm = ps.tile([128, 256], FP, tag="ps_big")
                    for k in range(4):
                        ptr = ps.tile([128, 128], BF, tag="ps_tr")
                        nc.tensor.transpose(ptr[:], perm[k][:, p * 128:(p + 1) * 128], identb[:])
                        scr = sp.tile([128, 128], BF, name=f"scr{k % 2}",
                                      tag=f"scr{k % 2}", bufs=2)
                        if k % 2 == 0:
                            nc.vector.tensor_copy(scr[:], ptr[:])
                        else:
                            nc.scalar.activation(scr[:], ptr[:], AF.Copy)
                        nc.tensor.matmul(ptm[:], scr[:], wbd[k][:],
                                         start=(k == 0), stop=(k == 3))
                    acc = sp.tile([128, 256], FP, tag="acc", bufs=2)
                    nc.vector.tensor_tensor(acc[:], ptm[:], biasf[:], op=OP.add)
                    dst = ob[:].rearrange("q (g t) -> q g t", g=G, t=T)[:, :, 4 * p:4 * p + 4]
                    src = acc[:].rearrange("q (g t) -> q g t", g=G, t=4)
                    nc.scalar.activation(dst, src, AF.Relu)
                nc.sync.dma_start(out_d[i * 128:(i + 1) * 128, :], ob[:])

    nc.compile()
    _compiled["nc"] = nc
    return nc


def _host_prep(x, edge_index, edge_weight, Ve, be, Vs, bs, cheb_W, cheb_b, res_W, res_b):
    import ml_dtypes
    row = np.asarray(edge_index[0]).astype(np.int64)
    col = np.asarray(edge_index[1]).astype(np.int64)
    w = np.asarray(edge_weight, np.float64).copy()
    w[row == col] = 0.0
    deg = np.zeros(N, np.float64)
    np.add.at(deg, row, w)
    dis = np.where(deg > 0, 1.0 / np.sqrt(np.where(deg > 0, deg, 1.0)), 0.0)
    norm = -dis[row] * w * dis[col]
    L = np.zeros((N, N), np.float64)
    np.add.at(L, (col, row), norm)

    cheb_W = np.asarray(cheb_W, np.float32)
    res_W = np.asarray(res_W, np.float32)
    wbd = np.zeros((4, 128, 256), np.float32)
    for tp in range(4):
        for k in range(3):
            wbd[k, tp * 32:(tp + 1) * 32, tp::4] = cheb_W[k]          # (F, G)
        wbd[3, tp * 32:(tp + 1) * 32, tp::4] = res_W.T                # (F, G)
    bias1 = (np.asarray(cheb_b, np.float32) + np.asarray(res_b, np.float32))
    biasf = np.repeat(np.repeat(bias1.reshape(1, G, 1), 4, axis=2).reshape(1, 256),
                      128, axis=0).astype(np.float32)

    return {
        "ident": np.eye(128, dtype=np.float32),
        "vet": np.ascontiguousarray(np.asarray(Ve, np.float32).T),
        "be": np.ascontiguousarray(np.asarray(be, np.float32)[0]),
        "vst": np.ascontiguousarray(np.asarray(Vs, np.float32).T).astype(ml_dtypes.bfloat16),
        "bs": np.ascontiguousarray(np.asarray(bs, np.float32)[0]),
        "lt": np.ascontiguousarray(L.T.astype(np.float32)).astype(ml_dtypes.bfloat16),
        "wbd": wbd.astype(ml_dtypes.bfloat16),
        "biasf": biasf,
    }


TRACE = False
LAST = {}


def kernel(x, edge_index, edge_weight, Ve, be, Vs, bs, cheb_W, cheb_b, res_W, res_b):
    from concourse.bass_utils import run_bass_kernel_spmd

    x = np.asarray(x, np.float32)
    shared = _host_prep(x, edge_index, edge_weight, Ve, be, Vs, bs,
                        cheb_W, cheb_b, res_W, res_b)
    nc = _build()
    in_maps = []
    for b in range(B):
        m = dict(shared)
        m["x"] = np.ascontiguousarray(x[b].reshape(N, D))
        in_maps.append(m)
    res = run_bass_kernel_spmd(nc, in_maps, list(range(B)), trace=TRACE)
    LAST["res"] = res
    out = np.stack([r["out"].reshape(N, G, T) for r in res.results], axis=0)
    return out



# revision 16
# speedup vs baseline: 2.4065x; 2.4065x over previous
"""STBlock (temporal attn -> spatial attn -> ChebConv + residual, relu) on 8 trn2 cores.

Sharding: data-parallel over batch B=8, one batch element per core.

v2 design notes (vs 509us baseline): the baseline burned ~93us of PE on 256
tiny 24-col transposes and ~300us of Vector/Scalar on per-instruction copy
overhead. This version:
  - uploads x from host in all three layouts it is consumed in (natural
    f-major, transposed f-major, transposed t-major), killing the stage-3
    transpose storm entirely;
  - keeps a t-major (d' = t*32+f) column order for every intermediate, so the
    final Cheb+residual projection is 12 plain 128-contract matmuls with
    block weights and zero permutes/transposes at the tail;
  - computes each Chebyshev propagation directly in transposed form
    (Z1^T = sum_m Z0[m,d'] * L^T[m,n]), halving transpose passes;
  - applies E_att via a banded 768x768 block-diagonal matmul (15 tile pairs)
    built on-device from eatt with quadrant-safe small copies;
  - folds the bs-add into the S_pre PSUM accumulation via an identity matmul,
    and skips softmax max-subtraction (scores are provably < ~5);
  - batches every PSUM->SBUF drain to >=384-col single instructions and
    round-robins them across Vector/GpSimd/Scalar.

Layouts (per core):
  d  = f*24+t (f-major), d' = t*32+f (t-major); out^T row = t*64+g.
  All partition offsets must be 32-aligned (BIR verifier quadrant rule), so
  f-blocks (24 rows/cols) are placed at 32-strides with zero padding.
  XNP[8]  (128n, 1024)  bf16   natural padded (col 32f+t), for score_t
  XT96P[8](128d+,1024n) bf16   x^T f-major padded (row 32j+u per 4-f group)
  XTT[6]  (128d',1024n) bf16   x^T t-major, residual rhs in projection
  TT96[8] (96d, 1024n)  bf16   x_TA^T compact f-major (E-mult out)
  AN[8]   (128n, 768d') bf16   x_TA natural t-major (transpose+permute of TT)
  SG[8]   (128n, 1024m) bf16   sigmoid(score_s)
  SATB    (128m, 8x1024n) bf16 S_att^T, m-tile blocks along free dim
  Z0T/Z1T/Z2T[6] (128d', 1024n) bf16; Z0N/Z1N[8] (128n, 768d') bf16
  out^T   (1536, 1024) bf16 -> host transposes back
"""
import numpy as np

B, N, F, T, G = 8, 1024, 32, 24, 64
D = F * T            # 768
NCH = N // 128       # 8 n-chunks
DCH = D // 128       # 6 d-tiles
QO = 12              # out^T tiles (1536 rows)

_compiled = {}


def _build():
    if "nc" in _compiled:
        return _compiled["nc"]
    import concourse.mybir as mybir
    import concourse.bacc as bacc
    from concourse import tile

    FP = mybir.dt.float32
    BF = mybir.dt.bfloat16
    AF = mybir.ActivationFunctionType
    OP = mybir.AluOpType

    nc = bacc.Bacc("TRN2", target_bir_lowering=False, debug=False)

    xnp_d = nc.dram_tensor("xnp", (N, 1024), BF, kind="ExternalInput").ap()
    xt96p_d = nc.dram_tensor("xt96p", (8 * 128, N), BF, kind="ExternalInput").ap()
    xtt_d = nc.dram_tensor("xtt", (D, N), BF, kind="ExternalInput").ap()
    identb_d = nc.dram_tensor("identb", (128, 128), BF, kind="ExternalInput").ap()
    vetb_d = nc.dram_tensor("vetb", (T, T), BF, kind="ExternalInput").ap()
    be_d = nc.dram_tensor("be", (T, T), FP, kind="ExternalInput").ap()
    vst_d = nc.dram_tensor("vst", (N, N), BF, kind="ExternalInput").ap()
    bst_d = nc.dram_tensor("bst", (N, N), BF, kind="ExternalInput").ap()
    lt_d = nc.dram_tensor("lt", (N, N), BF, kind="ExternalInput").ap()
    lt2_d = nc.dram_tensor("lt2", (N, N), BF, kind="ExternalInput").ap()
    wpb_d = nc.dram_tensor("wpb", (128, QO * 4 * 128), BF, kind="ExternalInput").ap()
    bias_d = nc.dram_tensor("bias128", (128, 1), FP, kind="ExternalInput").ap()
    out_d = nc.dram_tensor("out", (QO * 128, N), BF, kind="ExternalOutput").ap()

    with tile.TileContext(nc) as tc:
        with (
            tc.tile_pool(name="persist", bufs=1) as pp,
            tc.tile_pool(name="stream", bufs=1) as sp,
            tc.tile_pool(name="psb", bufs=2, space="PSUM") as psb,
            tc.tile_pool(name="pst", bufs=2, space="PSUM") as pst,
            tc.tile_pool(name="ps1", bufs=2, space="PSUM") as ps1,
        ):
            # round-robin for copy/cast work across DVE / Pool engines
            # (Act is kept for activations + a share of copies where idle)
            _rr = [0]
            PSUM_SPACE = tile.bass.MemorySpace.PSUM

            def copy_rr(dst, src, engines=None):
                if engines is None:
                    # GpSimd cannot touch PSUM
                    if src.space == PSUM_SPACE or dst.space == PSUM_SPACE:
                        engines = (nc.vector, nc.scalar)
                    else:
                        engines = (nc.vector, nc.gpsimd)
                e = engines[_rr[0] % len(engines)]
                _rr[0] += 1
                if e is nc.scalar:
                    nc.scalar.activation(dst, src, AF.Copy)
                else:
                    e.tensor_copy(dst, src)

            # ---- constants / inputs ----
            identb = pp.tile([128, 128], BF, tag="identb")
            nc.sync.dma_start(identb[:], identb_d[:])
            vetb = pp.tile([T, T], BF, tag="vetb")
            nc.sync.dma_start(vetb[:], vetb_d[:])
            be = pp.tile([T, T], FP, tag="be")
            nc.sync.dma_start(be[:], be_d[:])
            wpb = pp.tile([128, QO * 4 * 128], BF, tag="wpb")
            nc.sync.dma_start(wpb[:], wpb_d[:])
            bias128 = pp.tile([128, 1], FP, tag="bias128")
            nc.sync.dma_start(bias128[:], bias_d[:])

            XNP = []
            for i in range(NCH):
                t_ = pp.tile([128, 1024], BF, name=f"xnpA{i}", tag=f"A{i}")
                nc.sync.dma_start(t_[:], xnp_d[i * 128:(i + 1) * 128, :])
                XNP.append(t_)
            XT96P = []
            for g in range(8):
                t_ = pp.tile([128, N], BF, name=f"xt96B{g}", tag=f"B{g}")
                nc.sync.dma_start(t_[:], xt96p_d[g * 128:(g + 1) * 128, :])
                XT96P.append(t_)
            XTT = []
            for p in range(DCH):
                t_ = pp.tile([128, N], BF, name=f"xttX{p}", tag=f"X{p}")
                nc.sync.dma_start(t_[:], xtt_d[p * 128:(p + 1) * 128, :])
                XTT.append(t_)
            VST = []
            for m in range(NCH):
                t_ = pp.tile([128, N], BF, name=f"vstV{m}", tag=f"V{m}")
                nc.sync.dma_start(t_[:], vst_d[m * 128:(m + 1) * 128, :])
                VST.append(t_)

            # ---- S1: score_t = sum_{n,f} x[n,f,t] x[n,f,u] ----
            # XNP col blocks of 128 = 4 f's at 32-stride padding; the Gram of
            # each block has the per-f 24x24 diagonal blocks at 32-aligned
            # partition offsets. Garbage off-diagonal blocks are ignored.
            acc128 = pp.tile([128, 128], FP, tag="acc128")
            for g2 in range(8):
                pt = ps1.tile([128, 128], FP, tag="st")
                for i in range(NCH):
                    sl = XNP[i][:, g2 * 128:(g2 + 1) * 128]
                    nc.tensor.matmul(pt[:], sl, sl,
                                     start=(i == 0), stop=(i == NCH - 1))
                if g2 == 0:
                    nc.vector.tensor_copy(acc128[:], pt[:])
                else:
                    nc.vector.tensor_tensor(acc128[:], acc128[:], pt[:], op=OP.add)
            # TensorTensor needs equal base partitions for SBUF inputs, so
            # first move the three off-base diagonal blocks down to base 0.
            dg = []
            for j, eng in ((1, nc.vector), (2, nc.gpsimd), (3, nc.vector)):
                t_ = sp.tile([T, T], FP, name=f"dg{j}", tag=f"dg{j}")
                eng.tensor_copy(t_[:], acc128[32 * j:32 * j + 24,
                                              32 * j:32 * j + 24])
                dg.append(t_)
            sct_a = sp.tile([T, T], FP, tag="sct_a")
            nc.vector.tensor_tensor(sct_a[:], acc128[0:24, 0:24],
                                    dg[0][:], op=OP.add)
            sct_b = sp.tile([T, T], FP, tag="sct_b")
            nc.gpsimd.tensor_tensor(sct_b[:], dg[1][:], dg[2][:], op=OP.add)
            score_t = sp.tile([T, T], FP, tag="score_t")
            nc.vector.tensor_tensor(score_t[:], sct_a[:], sct_b[:], op=OP.add)

            # ---- S2: E_att = softmax(Ve @ sigmoid(score_t) + be) ----
            sigb = sp.tile([T, T], BF, tag="sigb")
            nc.scalar.activation(sigb[:], score_t[:], AF.Sigmoid)
            ps_e = ps1.tile([T, T], FP, tag="st")
            nc.tensor.matmul(ps_e[:], vetb[:], sigb[:], start=True, stop=True)
            epre = sp.tile([T, T], FP, tag="epre")
            nc.vector.tensor_tensor(epre[:], ps_e[:], be[:], op=OP.add)
            eexp = sp.tile([T, T], FP, tag="eexp")
            esum = sp.tile([T, 1], FP, tag="esum")
            nc.scalar.activation(eexp[:], epre[:], AF.Exp, accum_out=esum[:])
            einv = sp.tile([T, 1], FP, tag="einv")
            nc.vector.reciprocal(einv[:], esum[:])
            eatt = sp.tile([T, T], BF, tag="eatt")
            nc.vector.tensor_scalar_mul(eatt[:], eexp[:], einv[:])

            # e4: blockdiag(E_att x4) with 32-stride row padding (128, 96)
            e4 = pp.tile([128, 96], BF, tag="e4")
            nc.gpsimd.memset(e4[:], 0.0)
            for j in range(4):
                copy_rr(e4[32 * j:32 * j + 24, 24 * j:24 * j + 24], eatt[:])

            # ---- S3: TT96 = x_TA^T (compact f-major, 96-part tiles) ----
            TT96 = []
            for g in range(8):
                pb = psb.tile([96, N], FP, tag="big")
                for h in range(2):
                    nc.tensor.matmul(
                        pb[:, h * 512:(h + 1) * 512],
                        e4[:],
                        XT96P[g][:, h * 512:(h + 1) * 512],
                        start=True, stop=True)
                t_ = pp.tile([96, N], BF, name=f"ttT{g}", tag=f"T{g}")
                copy_rr(t_[:], pb[:])
                TT96.append(t_)

            # ---- S5 (score_s -> SG) interleaved with S4 (AN build) ----
            SG = []
            AN = []
            for i in range(NCH):
                pb = psb.tile([128, N], FP, tag="big")
                for h in range(2):
                    for g in range(8):
                        nc.tensor.matmul(
                            pb[:, h * 512:(h + 1) * 512],
                            TT96[g][:, i * 128:(i + 1) * 128],
                            TT96[g][:, h * 512:(h + 1) * 512],
                            start=(g == 0), stop=(g == 7))
                sg = pp.tile([128, N], BF, name=f"sgS{i}", tag=f"S{i}")
                nc.scalar.activation(sg[:], pb[:], AF.Sigmoid)
                SG.append(sg)

                pa = pst.tile([128, D], BF, tag="tr")
                for g in range(8):
                    nc.tensor.transpose(pa[:, g * 96:(g + 1) * 96],
                                        TT96[g][:, i * 128:(i + 1) * 128],
                                        identb[0:96, 0:96])
                an = pp.tile([128, D], BF, name=f"anN{i}", tag=f"AN{i}")
                # cols of pa: d = (4g+j)*24+u f-major; dst t-major d' = t*32+f
                copy_rr(an[:].rearrange("q (t f) -> q f t", t=T, f=F),
                        pa[:].rearrange("q (f t) -> q f t", f=F, t=T))
                AN.append(an)

            # ---- S6: S_att rows + softmax; S7: transpose into SATB ----
            satb = pp.tile([128, NCH * N], BF, tag="satb")
            for i in range(NCH):
                bsb = sp.tile([128, N], BF, tag="bsb", bufs=2)
                nc.sync.dma_start(bsb[:], bst_d[i * 128:(i + 1) * 128, :])
                pb = psb.tile([128, N], FP, tag="big")
                for h in range(2):
                    for m in range(NCH):
                        nc.tensor.matmul(
                            pb[:, h * 512:(h + 1) * 512],
                            VST[m][:, i * 128:(i + 1) * 128],
                            SG[m][:, h * 512:(h + 1) * 512],
                            start=(m == 0), stop=False)
                    nc.tensor.matmul(
                        pb[:, h * 512:(h + 1) * 512],
                        identb[:], bsb[:, h * 512:(h + 1) * 512],
                        start=False, stop=True)
                sexp = sp.tile([128, N], FP, tag="sexp", bufs=2)
                ssum = sp.tile([128, 1], FP, tag="ssum", bufs=2)
                nc.scalar.activation(sexp[:], pb[:], AF.Exp, accum_out=ssum[:])
                sinv = sp.tile([128, 1], FP, tag="sinv", bufs=2)
                nc.vector.reciprocal(sinv[:], ssum[:])
                sa = sp.tile([128, N], BF, tag="sa", bufs=2)
                nc.vector.tensor_scalar_mul(sa[:], sexp[:], sinv[:])
                for grp in range(2):
                    pq = pst.tile([128, 512], BF, tag="tr")
                    for k in range(4):
                        p = grp * 4 + k
                        nc.tensor.transpose(pq[:, k * 128:(k + 1) * 128],
                                            sa[:, p * 128:(p + 1) * 128],
                                            identb[:])
                    dst = satb[:].rearrange("q (p n) -> q p n", p=NCH)[
                        :, grp * 4:grp * 4 + 4, i * 128:(i + 1) * 128]
                    copy_rr(dst, pq[:].rearrange("q (p n) -> q p n", p=4))

            # LT into SG slots (SG dead), LT2 into B slots + 2 fresh
            LT = []
            for m in range(NCH):
                t_ = pp.tile([128, N], BF, name=f"ltS{m}", tag=f"S{m}")
                nc.sync.dma_start(t_[:], lt_d[m * 128:(m + 1) * 128, :])
                LT.append(t_)
            LT2 = []
            for m in range(NCH):
                t_ = pp.tile([128, N], BF, name=f"lt2{m}", tag=f"B{m}")
                nc.sync.dma_start(t_[:], lt2_d[m * 128:(m + 1) * 128, :])
                LT2.append(t_)

            # ---- S8: Z0T = (S_att @ x_TA)^T directly (t-major) ----
            Z0T = []
            for p in range(DCH):
                pb = psb.tile([128, N], FP, tag="big")
                for h in range(2):
                    for m in range(NCH):
                        nc.tensor.matmul(
                            pb[:, h * 512:(h + 1) * 512],
                            AN[m][:, p * 128:(p + 1) * 128],
                            satb[:, m * N + h * 512:m * N + (h + 1) * 512],
                            start=(m == 0), stop=(m == NCH - 1))
                t_ = pp.tile([128, N], BF, name=f"z0tT{p}", tag=f"T{p}")
                copy_rr(t_[:], pb[:])
                Z0T.append(t_)

            # ---- S9: Z0N = transpose(Z0T) ----
            Z0N = []
            for i in range(NCH):
                z = pp.tile([128, D], BF, name=f"z0nA{i}", tag=f"A{i}")
                Z0N.append(z)
            for i in range(NCH):
                for grp in range(2):
                    pz = pst.tile([128, 384], BF, tag="tr")
                    for k in range(3):
                        p = grp * 3 + k
                        nc.tensor.transpose(pz[:, k * 128:(k + 1) * 128],
                                            Z0T[p][:, i * 128:(i + 1) * 128],
                                            identb[:])
                    copy_rr(Z0N[i][:, grp * 384:(grp + 1) * 384], pz[:])

            # ---- S10: Z1T[d',n] = sum_m Z0[m,d'] L^T[m,n] = (L@Z0)^T ----
            Z1T = []
            for p in range(DCH):
                pb = psb.tile([128, N], FP, tag="big")
                for h in range(2):
                    for m in range(NCH):
                        nc.tensor.matmul(
                            pb[:, h * 512:(h + 1) * 512],
                            Z0N[m][:, p * 128:(p + 1) * 128],
                            LT[m][:, h * 512:(h + 1) * 512],
                            start=(m == 0), stop=(m == NCH - 1))
                t_ = pp.tile([128, N], BF, name=f"z1tV{p}", tag=f"V{p}")
                copy_rr(t_[:], pb[:])
                Z1T.append(t_)

            # ---- S11: Z1N = transpose(Z1T) ----
            Z1N = []
            for i in range(NCH):
                z = pp.tile([128, D], BF, name=f"z1nN{i}", tag=f"AN{i}")
                Z1N.append(z)
            for i in range(NCH):
                for grp in range(2):
                    pz = pst.tile([128, 384], BF, tag="tr")
                    for k in range(3):
                        p = grp * 3 + k
                        nc.tensor.transpose(pz[:, k * 128:(k + 1) * 128],
                                            Z1T[p][:, i * 128:(i + 1) * 128],
                                            identb[:])
                    copy_rr(Z1N[i][:, grp * 384:(grp + 1) * 384], pz[:])

            # ---- S12: Z2T = 2*(L@Z1)^T - Z0T ----
            Z2T = []
            for p in range(DCH):
                pb = psb.tile([128, N], FP, tag="big")
                for h in range(2):
                    for m in range(NCH):
                        nc.tensor.matmul(
                            pb[:, h * 512:(h + 1) * 512],
                            Z1N[m][:, p * 128:(p + 1) * 128],
                            LT2[m][:, h * 512:(h + 1) * 512],
                            start=(m == 0), stop=(m == NCH - 1))
                t_ = pp.tile([128, N], BF, name=f"z2tZ{p}", tag=f"Z2{p}")
                if p % 2 == 0:
                    nc.vector.tensor_tensor(t_[:], pb[:], Z0T[p][:], op=OP.subtract)
                else:
                    # spread load: Act drains PSUM, Pool does the SBUF subtract
                    pc = sp.tile([128, N], BF, tag="z2c", bufs=2)
                    nc.scalar.activation(pc[:], pb[:], AF.Copy)
                    nc.gpsimd.tensor_tensor(t_[:], pc[:], Z0T[p][:], op=OP.subtract)
                Z2T.append(t_)

            # ---- S13: projection (Cheb k=0..2 + residual), bias, relu ----
            for q in range(QO):
                p = q // 2
                pb = psb.tile([128, N], FP, tag="big")
                rhs4 = (Z0T[p], Z1T[p], Z2T[p], XTT[p])
                for h in range(2):
                    for k in range(4):
                        nc.tensor.matmul(
                            pb[:, h * 512:(h + 1) * 512],
                            wpb[:, (4 * q + k) * 128:(4 * q + k + 1) * 128],
                            rhs4[k][:, h * 512:(h + 1) * 512],
                            start=(k == 0), stop=(k == 3))
                ob = sp.tile([128, N], BF, tag="outbuf", bufs=2)
                nc.scalar.activation(ob[:], pb[:], AF.Relu, bias=bias128[:])
                nc.sync.dma_start(out_d[q * 128:(q + 1) * 128, :], ob[:])

    nc.compile()
    _compiled["nc"] = nc
    return nc


def _host_prep(x, edge_index, edge_weight, Ve, be, Vs, bs, cheb_W, cheb_b, res_W, res_b):
    import ml_dtypes
    BF = ml_dtypes.bfloat16
    row = np.asarray(edge_index[0]).astype(np.int64)
    col = np.asarray(edge_index[1]).astype(np.int64)
    w = np.asarray(edge_weight, np.float64).copy()
    w[row == col] = 0.0
    deg = np.zeros(N, np.float64)
    np.add.at(deg, row, w)
    dis = np.where(deg > 0, 1.0 / np.sqrt(np.where(deg > 0, deg, 1.0)), 0.0)
    norm = -dis[row] * w * dis[col]
    L = np.zeros((N, N), np.float64)
    np.add.at(L, (col, row), norm)
    LT = np.ascontiguousarray(L.T.astype(np.float32))

    cheb_W = np.asarray(cheb_W, np.float32)
    res_W = np.asarray(res_W, np.float32)
    # wpb[p, (4q+k)*128 + c] = blk(q,k)[p, c]; out^T tile q rows (t,g) with
    # t = 2q + c//64, contracting t-major tile p=q//2 rows (t', f)
    wq = np.zeros((QO, 4, 128, 128), np.float32)
    Wlist = [cheb_W[0], cheb_W[1], cheb_W[2], res_W.T]  # each (F, G)
    for q in range(QO):
        off = 0 if q % 2 == 0 else 2
        for b_ in range(2):
            a = b_ + off
            for k in range(4):
                wq[q, k, 32 * a:32 * a + 32, 64 * b_:64 * b_ + 64] = Wlist[k]
    wpb = np.ascontiguousarray(
        wq.transpose(2, 0, 1, 3).reshape(128, QO * 4 * 128)).astype(BF)

    b64 = (np.asarray(cheb_b, np.float32) + np.asarray(res_b, np.float32))
    bias128 = np.concatenate([b64, b64]).reshape(128, 1).astype(np.float32)

    return {
        "identb": np.eye(128, dtype=np.float32).astype(BF),
        "vetb": np.ascontiguousarray(np.asarray(Ve, np.float32).T).astype(BF),
        "be": np.ascontiguousarray(np.asarray(be, np.float32)[0]),
        "vst": np.ascontiguousarray(np.asarray(Vs, np.float32).T).astype(BF),
        "bst": np.ascontiguousarray(np.asarray(bs, np.float32)[0]).astype(BF),
        "lt": LT.astype(BF),
        "lt2": (2.0 * LT).astype(BF),
        "wpb": wpb,
        "bias128": bias128,
    }


TRACE = False
LAST = {}


def kernel(x, edge_index, edge_weight, Ve, be, Vs, bs, cheb_W, cheb_b, res_W, res_b):
    from concourse.bass_utils import run_bass_kernel_spmd
    import ml_dtypes
    BF = ml_dtypes.bfloat16

    x = np.asarray(x, np.float32)
    shared = _host_prep(x, edge_index, edge_weight, Ve, be, Vs, bs,
                        cheb_W, cheb_b, res_W, res_b)
    nc = _build()
    in_maps = []
    for b in range(B):
        m = dict(shared)
        xb = x[b]                                   # (N, F, T)
        xnp = np.zeros((N, F, 32), np.float32)      # col 32f+t, zero padded
        xnp[:, :, :T] = xb
        m["xnp"] = np.ascontiguousarray(xnp.reshape(N, 1024)).astype(BF)
        xtf = xb.reshape(N, D).T                    # (768, N), d = f*24+t
        x96p = np.zeros((8, 4, 32, N), np.float32)  # row 128g+32j+u
        x96p[:, :, :T, :] = xtf.reshape(8, 4, T, N)
        m["xt96p"] = np.ascontiguousarray(x96p.reshape(8 * 128, N)).astype(BF)
        m["xtt"] = np.ascontiguousarray(
            xb.transpose(2, 1, 0).reshape(D, N)).astype(BF)  # d' = t*32+f
        in_maps.append(m)
    res = run_bass_kernel_spmd(nc, in_maps, list(range(B)), trace=TRACE)
    LAST["res"] = res
    out = np.stack(
        [r["out"].astype(np.float32).reshape(T, G, N).transpose(2, 1, 0)
         for r in res.results], axis=0)
    return out


# revision 28
# speedup vs baseline: 2.6161x; 1.0871x over previous
"""STBlock (temporal attn -> spatial attn -> ChebConv + residual, relu) on 8 trn2 cores.

Sharding: data-parallel over batch B=8, one batch element per core.

v2 design notes (vs 509us baseline): the baseline burned ~93us of PE on 256
tiny 24-col transposes and ~300us of Vector/Scalar on per-instruction copy
overhead. This version:
  - uploads x from host in all three layouts it is consumed in (natural
    f-major, transposed f-major, transposed t-major), killing the stage-3
    transpose storm entirely;
  - keeps a t-major (d' = t*32+f) column order for every intermediate, so the
    final Cheb+residual projection is 12 plain 128-contract matmuls with
    block weights and zero permutes/transposes at the tail;
  - computes each Chebyshev propagation directly in transposed form
    (Z1^T = sum_m Z0[m,d'] * L^T[m,n]), halving transpose passes;
  - applies E_att via a banded 768x768 block-diagonal matmul (15 tile pairs)
    built on-device from eatt with quadrant-safe small copies;
  - folds the bs-add into the S_pre PSUM accumulation via an identity matmul,
    and skips softmax max-subtraction (scores are provably < ~5);
  - batches every PSUM->SBUF drain to >=384-col single instructions and
    round-robins them across Vector/GpSimd/Scalar.

Layouts (per core):
  d  = f*24+t (f-major), d' = t*32+f (t-major); out^T row = t*64+g.
  All partition offsets must be 32-aligned (BIR verifier quadrant rule), so
  f-blocks (24 rows/cols) are placed at 32-strides with zero padding.
  XNP[8]  (128n, 1024)  bf16   natural padded (col 32f+t), for score_t
  XT96P[8](128d+,1024n) bf16   x^T f-major padded (row 32j+u per 4-f group)
  XTT[6]  (128d',1024n) bf16   x^T t-major, residual rhs in projection
  TT96[8] (96d, 1024n)  bf16   x_TA^T compact f-major (E-mult out)
  AN[8]   (128n, 768d') bf16   x_TA natural t-major (transpose+permute of TT)
  SG[8]   (128n, 1024m) bf16   sigmoid(score_s)
  SATB    (128m, 8x1024n) bf16 S_att^T, m-tile blocks along free dim
  Z0T/Z1T/Z2T[6] (128d', 1024n) bf16; Z0N/Z1N[8] (128n, 768d') bf16
  out^T   (1536, 1024) bf16 -> host transposes back
"""
import numpy as np

B, N, F, T, G = 8, 1024, 32, 24, 64
D = F * T            # 768
NCH = N // 128       # 8 n-chunks
DCH = D // 128       # 6 d-tiles
QO = 12              # out^T tiles (1536 rows)

_compiled = {}


def _build():
    if "nc" in _compiled:
        return _compiled["nc"]
    import concourse.mybir as mybir
    import concourse.bacc as bacc
    from concourse import tile

    FP = mybir.dt.float32
    BF = mybir.dt.bfloat16
    F8 = mybir.dt.float8e4
    AF = mybir.ActivationFunctionType
    OP = mybir.AluOpType
    DR = mybir.MatmulPerfMode.DoubleRow

    nc = bacc.Bacc("TRN2", target_bir_lowering=False, debug=False)

    xnp_d = nc.dram_tensor("xnp", (N, 1024), BF, kind="ExternalInput").ap()
    xt96p_d = nc.dram_tensor("xt96p", (8 * 128, N), BF, kind="ExternalInput").ap()
    xtt_d = nc.dram_tensor("xtt", (D, N), BF, kind="ExternalInput").ap()
    identb_d = nc.dram_tensor("identb", (128, 128), BF, kind="ExternalInput").ap()
    ident8_d = nc.dram_tensor("ident8", (128, 128), F8, kind="ExternalInput").ap()
    vetb_d = nc.dram_tensor("vetb", (T, T), BF, kind="ExternalInput").ap()
    be_d = nc.dram_tensor("be", (T, T), FP, kind="ExternalInput").ap()
    vst_d = nc.dram_tensor("vst8", (N, N), F8, kind="ExternalInput").ap()
    bst_d = nc.dram_tensor("bst", (N, N), BF, kind="ExternalInput").ap()
    lt_d = nc.dram_tensor("lt", (N, N), BF, kind="ExternalInput").ap()
    lt2_d = nc.dram_tensor("lt2", (N, N), BF, kind="ExternalInput").ap()
    wpb_d = nc.dram_tensor("wpb", (128, QO * 4 * 128), BF, kind="ExternalInput").ap()
    bias_d = nc.dram_tensor("bias128", (128, 1), FP, kind="ExternalInput").ap()
    out_d = nc.dram_tensor("out", (QO * 128, N), BF, kind="ExternalOutput").ap()

    with tile.TileContext(nc) as tc:
        with (
            tc.tile_pool(name="persist", bufs=1) as pp,
            tc.tile_pool(name="stream", bufs=1) as sp,
            tc.tile_pool(name="psb", bufs=2, space="PSUM") as psb,
            tc.tile_pool(name="pst", bufs=2, space="PSUM") as pst,
            tc.tile_pool(name="ps1", bufs=2, space="PSUM") as ps1,
        ):
            # round-robin for copy/cast work across DVE / Pool engines
            # (Act is kept for activations + a share of copies where idle)
            _rr = [0]
            PSUM_SPACE = tile.bass.MemorySpace.PSUM

            def copy_rr(dst, src, engines=None):
                if engines is None:
                    # GpSimd cannot touch PSUM
                    if src.space == PSUM_SPACE or dst.space == PSUM_SPACE:
                        engines = (nc.vector, nc.scalar)
                    else:
                        engines = (nc.vector, nc.gpsimd)
                e = engines[_rr[0] % len(engines)]
                _rr[0] += 1
                if e is nc.scalar:
                    nc.scalar.activation(dst, src, AF.Copy)
                else:
                    e.tensor_copy(dst, src)

            # ---- constants / inputs ----
            identb = pp.tile([128, 128], BF, tag="identb")
            nc.sync.dma_start(identb[:], identb_d[:])
            ident8 = pp.tile([128, 128], F8, tag="ident8")
            nc.sync.dma_start(ident8[:], ident8_d[:])
            vetb = pp.tile([T, T], BF, tag="vetb")
            nc.sync.dma_start(vetb[:], vetb_d[:])
            be = pp.tile([T, T], FP, tag="be")
            nc.sync.dma_start(be[:], be_d[:])
            # preload Act function tables off the critical path
            warm = sp.tile([1, 1], FP, tag="warm")
            nc.scalar.activation(warm[:], identb[0:1, 0:1], AF.Sigmoid)
            nc.scalar.activation(warm[:], identb[0:1, 0:1], AF.Exp)
            nc.scalar.activation(warm[:], identb[0:1, 0:1], AF.Relu)
            wpb = pp.tile([128, QO * 4 * 128], BF, tag="wpb")
            nc.sync.dma_start(wpb[:], wpb_d[:])
            bias128 = pp.tile([128, 1], FP, tag="bias128")
            nc.sync.dma_start(bias128[:], bias_d[:])

            XNP = []
            for i in range(NCH):
                t_ = pp.tile([128, 1024], BF, name=f"xnpA{i}", tag=f"A{i}")
                nc.sync.dma_start(t_[:], xnp_d[i * 128:(i + 1) * 128, :])
                XNP.append(t_)
            XT96P = []
            for g in range(8):
                t_ = pp.tile([128, N], BF, name=f"xt96B{g}", tag=f"B{g}")
                nc.sync.dma_start(t_[:], xt96p_d[g * 128:(g + 1) * 128, :])
                XT96P.append(t_)
            XTT = []
            for p in range(DCH):
                t_ = pp.tile([128, N], BF, name=f"xttX{p}", tag=f"X{p}")
                nc.sync.dma_start(t_[:], xtt_d[p * 128:(p + 1) * 128, :])
                XTT.append(t_)
            # Vs^T as one (128, 8x1024) fp8 tile: col block m holds m-tile rows
            vst8 = pp.tile([128, NCH * N], F8, tag="vst8")
            for m in range(NCH):
                nc.sync.dma_start(vst8[:, m * N:(m + 1) * N],
                                  vst_d[m * 128:(m + 1) * 128, :])
            vst8v = vst8[:].rearrange("q (m n) -> q m n", m=NCH)

            # ---- S1: score_t = sum_{n,f} x[n,f,t] x[n,f,u] ----
            # XNP col blocks of 128 = 4 f's at 32-stride padding; the Gram of
            # each block has the per-f 24x24 diagonal blocks at 32-aligned
            # partition offsets. Garbage off-diagonal blocks are ignored.
            acc128 = pp.tile([128, 128], FP, tag="acc128")
            for g2 in range(8):
                pt = ps1.tile([128, 128], FP, tag="st")
                for i in range(NCH):
                    sl = XNP[i][:, g2 * 128:(g2 + 1) * 128]
                    nc.tensor.matmul(pt[:], sl, sl,
                                     start=(i == 0), stop=(i == NCH - 1))
                if g2 == 0:
                    nc.vector.tensor_copy(acc128[:], pt[:])
                else:
                    nc.vector.tensor_tensor(acc128[:], acc128[:], pt[:], op=OP.add)
            # TensorTensor needs equal base partitions for SBUF inputs, so
            # first move the three off-base diagonal blocks down to base 0.
            dg = []
            for j, eng in ((1, nc.vector), (2, nc.gpsimd), (3, nc.vector)):
                t_ = sp.tile([T, T], FP, name=f"dg{j}", tag=f"dg{j}")
                eng.tensor_copy(t_[:], acc128[32 * j:32 * j + 24,
                                              32 * j:32 * j + 24])
                dg.append(t_)
            sct_a = sp.tile([T, T], FP, tag="sct_a")
            nc.vector.tensor_tensor(sct_a[:], acc128[0:24, 0:24],
                                    dg[0][:], op=OP.add)
            sct_b = sp.tile([T, T], FP, tag="sct_b")
            nc.gpsimd.tensor_tensor(sct_b[:], dg[1][:], dg[2][:], op=OP.add)
            score_t = sp.tile([T, T], FP, tag="score_t")
            nc.vector.tensor_tensor(score_t[:], sct_a[:], sct_b[:], op=OP.add)

            # ---- S2: E_att = softmax(Ve @ sigmoid(score_t) + be) ----
            sigb = sp.tile([T, T], BF, tag="sigb")
            nc.scalar.activation(sigb[:], score_t[:], AF.Sigmoid)
            ps_e = ps1.tile([T, T], FP, tag="st")
            nc.tensor.matmul(ps_e[:], vetb[:], sigb[:], start=True, stop=True)
            epre = sp.tile([T, T], FP, tag="epre")
            nc.vector.tensor_tensor(epre[:], ps_e[:], be[:], op=OP.add)
            eexp = sp.tile([T, T], FP, tag="eexp")
            esum = sp.tile([T, 1], FP, tag="esum")
            nc.scalar.activation(eexp[:], epre[:], AF.Exp, accum_out=esum[:])
            einv = sp.tile([T, 1], FP, tag="einv")
            nc.vector.reciprocal(einv[:], esum[:])
            eatt = sp.tile([T, T], BF, tag="eatt")
            nc.vector.tensor_scalar_mul(eatt[:], eexp[:], einv[:])

            # e4: blockdiag(E_att x4) with 32-stride row padding (128, 96)
            e4 = pp.tile([128, 96], BF, tag="e4")
            nc.gpsimd.memset(e4[:], 0.0)
            for j in range(4):
                copy_rr(e4[32 * j:32 * j + 24, 24 * j:24 * j + 24], eatt[:])

            # ---- S3: TT8 = x_TA^T (f-major), packed into 128-row d-tiles fp8
            # col block p of TT8 holds d-tile p (for DoubleRow pairing).
            tt8 = pp.tile([128, DCH * N], F8, tag="tt8")
            for g in range(8):
                pb = psb.tile([96, N], FP, tag="big")
                for h in range(2):
                    nc.tensor.matmul(
                        pb[:, h * 512:(h + 1) * 512],
                        e4[:],
                        XT96P[g][:, h * 512:(h + 1) * 512],
                        start=True, stop=True)
                a = 96 * g
                while a < 96 * (g + 1):
                    p, rloc = divmod(a, 128)
                    seg = min(96 * (g + 1) - a, 128 - rloc)
                    if rloc != 0 or (a - 96 * g) != 0:
                        seg = min(seg, 32)  # quadrant rule for offset APs
                    copy_rr(tt8[rloc:rloc + seg, p * N:(p + 1) * N],
                            pb[a - 96 * g:a - 96 * g + seg, :])
                    a += seg
            tt8v = tt8[:].rearrange("q (p n) -> q p n", p=DCH)

            # ---- S5 (score_s -> SG) interleaved with S4 (AN build) ----
            sg8 = pp.tile([128, NCH * N], F8, tag="sg8")
            sg8v = sg8[:].rearrange("q (m n) -> q m n", m=NCH)
            anb = pp.tile([128, NCH * D], F8, tag="anb")
            anbv = anb[:].rearrange("q (m d) -> q m d", m=NCH)
            for i in range(NCH):
                pb = psb.tile([128, N], FP, tag="big")
                for h in range(2):
                    for a2 in range(3):  # DoubleRow over d-tile pairs
                        nc.tensor.matmul(
                            pb[:, h * 512:(h + 1) * 512],
                            tt8v[:, 2 * a2:2 * a2 + 2, i * 128:(i + 1) * 128],
                            tt8v[:, 2 * a2:2 * a2 + 2, h * 512:(h + 1) * 512],
                            start=(a2 == 0), stop=(a2 == 2), perf_mode=DR)
                nc.scalar.activation(sg8[:, i * N:(i + 1) * N], pb[:], AF.Sigmoid)

                # fp8 transpose must write psum with element step 2
                pa = pst.tile([128, 2 * D], F8, tag="tr")
                pav = pa[:].rearrange("q (c two) -> q two c", two=2)
                for p in range(DCH):
                    nc.tensor.transpose(pav[:, 0, p * 128:(p + 1) * 128],
                                        tt8[:, p * N + i * 128:p * N + (i + 1) * 128],
                                        ident8[:])
                dense = sp.tile([128, D], F8, tag="dense", bufs=2)
                copy_rr(dense[:].unsqueeze(2),
                        pa[:].rearrange("q (c two) -> q c two", two=2)[:, :, 0:1])
                # cols of dense: d = f*24+t f-major; dst t-major d' = t*32+f
                copy_rr(anb[:, i * D:(i + 1) * D].rearrange(
                            "q (t f) -> q f t", t=T, f=F),
                        dense[:].rearrange("q (f t) -> q f t", f=F, t=T))

            # ---- S6: S_att rows + softmax; S7: transpose into SATB ----
            satb = pp.tile([128, NCH * N], F8, tag="satb")
            satbv = satb[:].rearrange("q (m n) -> q m n", m=NCH)
            for i in range(NCH):
                bsb = sp.tile([128, N], BF, tag="bsb", bufs=2)
                nc.sync.dma_start(bsb[:], bst_d[i * 128:(i + 1) * 128, :])
                pb = psb.tile([128, N], FP, tag="big")
                for h in range(2):
                    for a2 in range(4):  # DoubleRow over m-tile pairs
                        nc.tensor.matmul(
                            pb[:, h * 512:(h + 1) * 512],
                            vst8v[:, 2 * a2:2 * a2 + 2, i * 128:(i + 1) * 128],
                            sg8v[:, 2 * a2:2 * a2 + 2, h * 512:(h + 1) * 512],
                            start=(a2 == 0), stop=False, perf_mode=DR)
                    nc.tensor.matmul(
                        pb[:, h * 512:(h + 1) * 512],
                        identb[:], bsb[:, h * 512:(h + 1) * 512],
                        start=False, stop=True)
                sexp = sp.tile([128, N], FP, tag="sexp", bufs=2)
                ssum = sp.tile([128, 1], FP, tag="ssum", bufs=2)
                nc.scalar.activation(sexp[:], pb[:], AF.Exp, accum_out=ssum[:])
                sinv = sp.tile([128, 1], FP, tag="sinv", bufs=2)
                nc.vector.reciprocal(sinv[:], ssum[:])
                # x512 keeps softmax weights above the fp8e4m3 subnormal floor;
                # the Z0T drain divides it back out.
                sa = sp.tile([128, N], BF, tag="sa", bufs=2)
                nc.vector.tensor_scalar(sa[:], sexp[:], sinv[:], 512.0,
                                        op0=OP.mult, op1=OP.mult)
                for grp in range(2):
                    pq = pst.tile([128, 512], BF, tag="tr")
                    for k in range(4):
                        p = grp * 4 + k
                        nc.tensor.transpose(pq[:, k * 128:(k + 1) * 128],
                                            sa[:, p * 128:(p + 1) * 128],
                                            identb[:])
                    dst = satb[:].rearrange("q (p n) -> q p n", p=NCH)[
                        :, grp * 4:grp * 4 + 4, i * 128:(i + 1) * 128]
                    copy_rr(dst, pq[:].rearrange("q (p n) -> q p n", p=4))

            # LT into SG slots (SG dead), LT2 into B slots + 2 fresh
            LT = []
            for m in range(NCH):
                t_ = pp.tile([128, N], BF, name=f"ltS{m}", tag=f"S{m}")
                nc.sync.dma_start(t_[:], lt_d[m * 128:(m + 1) * 128, :])
                LT.append(t_)
            LT2 = []
            for m in range(NCH):
                t_ = pp.tile([128, N], BF, name=f"lt2{m}", tag=f"B{m}")
                nc.sync.dma_start(t_[:], lt2_d[m * 128:(m + 1) * 128, :])
                LT2.append(t_)

            # ---- S8: Z0T = (S_att @ x_TA)^T directly (t-major) ----
            Z0T = []
            for p in range(DCH):
                pb = psb.tile([128, N], FP, tag="big")
                for h in range(2):
                    for a2 in range(4):  # DoubleRow over m-tile pairs
                        nc.tensor.matmul(
                            pb[:, h * 512:(h + 1) * 512],
                            anbv[:, 2 * a2:2 * a2 + 2, p * 128:(p + 1) * 128],
                            satbv[:, 2 * a2:2 * a2 + 2, h * 512:(h + 1) * 512],
                            start=(a2 == 0), stop=(a2 == 3), perf_mode=DR)
                t_ = pp.tile([128, N], BF, name=f"z0tT{p}", tag=f"T{p}")
                if p % 2 == 0:
                    nc.vector.tensor_scalar_mul(t_[:], pb[:], 1.0 / 512.0)
                else:
                    nc.scalar.activation(t_[:], pb[:], AF.Copy, scale=1.0 / 512.0)
                Z0T.append(t_)

            # ---- S9: Z0N = transpose(Z0T) ----
            Z0N = []
            for i in range(NCH):
                z = pp.tile([128, D], BF, name=f"z0nA{i}", tag=f"A{i}")
                Z0N.append(z)
            for i in range(NCH):
                for grp in range(2):
                    pz = pst.tile([128, 384], BF, tag="tr")
                    for k in range(3):
                        p = grp * 3 + k
                        nc.tensor.transpose(pz[:, k * 128:(k + 1) * 128],
                                            Z0T[p][:, i * 128:(i + 1) * 128],
                                            identb[:])
                    copy_rr(Z0N[i][:, grp * 384:(grp + 1) * 384], pz[:])

            # ---- S10: Z1T[d',n] = sum_m Z0[m,d'] L^T[m,n] = (L@Z0)^T ----
            Z1T = []
            for p in range(DCH):
                pb = psb.tile([128, N], FP, tag="big")
                for h in range(2):
                    for m in range(NCH):
                        nc.tensor.matmul(
                            pb[:, h * 512:(h + 1) * 512],
                            Z0N[m][:, p * 128:(p + 1) * 128],
                            LT[m][:, h * 512:(h + 1) * 512],
                            start=(m == 0), stop=(m == NCH - 1))
                t_ = pp.tile([128, N], BF, name=f"z1tV{p}", tag=f"V{p}")
                copy_rr(t_[:], pb[:])
                Z1T.append(t_)

            # ---- S11: Z1N = transpose(Z1T) ----
            Z1N = []
            for i in range(NCH):
                z = pp.tile([128, D], BF, name=f"z1nN{i}", tag=f"AN{i}")
                Z1N.append(z)
            for i in range(NCH):
                for grp in range(2):
                    pz = pst.tile([128, 384], BF, tag="tr")
                    for k in range(3):
                        p = grp * 3 + k
                        nc.tensor.transpose(pz[:, k * 128:(k + 1) * 128],
                                            Z1T[p][:, i * 128:(i + 1) * 128],
                                            identb[:])
                    copy_rr(Z1N[i][:, grp * 384:(grp + 1) * 384], pz[:])

            # ---- S12: Z2T = 2*(L@Z1)^T - Z0T ----
            Z2T = []
            for p in range(DCH):
                pb = psb.tile([128, N], FP, tag="big")
                for h in range(2):
                    for m in range(NCH):
                        nc.tensor.matmul(
                            pb[:, h * 512:(h + 1) * 512],
                            Z1N[m][:, p * 128:(p + 1) * 128],
                            LT2[m][:, h * 512:(h + 1) * 512],
                            start=(m == 0), stop=(m == NCH - 1))
                t_ = pp.tile([128, N], BF, name=f"z2tZ{p}", tag=f"Z2{p}")
                if p % 2 == 0:
                    nc.vector.tensor_tensor(t_[:], pb[:], Z0T[p][:], op=OP.subtract)
                else:
                    # spread load: Act drains PSUM, Pool does the SBUF subtract
                    pc = sp.tile([128, N], BF, tag="z2c", bufs=2)
                    nc.scalar.activation(pc[:], pb[:], AF.Copy)
                    nc.gpsimd.tensor_tensor(t_[:], pc[:], Z0T[p][:], op=OP.subtract)
                Z2T.append(t_)

            # ---- S13: projection (Cheb k=0..2 + residual), bias, relu ----
            for q in range(QO):
                p = q // 2
                pb = psb.tile([128, N], FP, tag="big")
                rhs4 = (Z0T[p], Z1T[p], Z2T[p], XTT[p])
                for h in range(2):
                    for k in range(4):
                        nc.tensor.matmul(
                            pb[:, h * 512:(h + 1) * 512],
                            wpb[:, (4 * q + k) * 128:(4 * q + k + 1) * 128],
                            rhs4[k][:, h * 512:(h + 1) * 512],
                            start=(k == 0), stop=(k == 3))
                ob = sp.tile([128, N], BF, tag="outbuf", bufs=2)
                nc.scalar.activation(ob[:], pb[:], AF.Relu, bias=bias128[:])
                nc.sync.dma_start(out_d[q * 128:(q + 1) * 128, :], ob[:])

    nc.compile()
    _compiled["nc"] = nc
    return nc


def _host_prep(x, edge_index, edge_weight, Ve, be, Vs, bs, cheb_W, cheb_b, res_W, res_b):
    import ml_dtypes
    BF = ml_dtypes.bfloat16
    row = np.asarray(edge_index[0]).astype(np.int64)
    col = np.asarray(edge_index[1]).astype(np.int64)
    w = np.asarray(edge_weight, np.float64).copy()
    w[row == col] = 0.0
    deg = np.zeros(N, np.float64)
    np.add.at(deg, row, w)
    dis = np.where(deg > 0, 1.0 / np.sqrt(np.where(deg > 0, deg, 1.0)), 0.0)
    norm = -dis[row] * w * dis[col]
    L = np.zeros((N, N), np.float64)
    np.add.at(L, (col, row), norm)
    LT = np.ascontiguousarray(L.T.astype(np.float32))

    cheb_W = np.asarray(cheb_W, np.float32)
    res_W = np.asarray(res_W, np.float32)
    # wpb[p, (4q+k)*128 + c] = blk(q,k)[p, c]; out^T tile q rows (t,g) with
    # t = 2q + c//64, contracting t-major tile p=q//2 rows (t', f)
    wq = np.zeros((QO, 4, 128, 128), np.float32)
    Wlist = [cheb_W[0], cheb_W[1], cheb_W[2], res_W.T]  # each (F, G)
    for q in range(QO):
        off = 0 if q % 2 == 0 else 2
        for b_ in range(2):
            a = b_ + off
            for k in range(4):
                wq[q, k, 32 * a:32 * a + 32, 64 * b_:64 * b_ + 64] = Wlist[k]
    wpb = np.ascontiguousarray(
        wq.transpose(2, 0, 1, 3).reshape(128, QO * 4 * 128)).astype(BF)

    b64 = (np.asarray(cheb_b, np.float32) + np.asarray(res_b, np.float32))
    bias128 = np.concatenate([b64, b64]).reshape(128, 1).astype(np.float32)

    import ml_dtypes as mld
    return {
        "identb": np.eye(128, dtype=np.float32).astype(BF),
        "ident8": np.eye(128, dtype=np.float32).astype(mld.float8_e4m3),
        "vetb": np.ascontiguousarray(np.asarray(Ve, np.float32).T).astype(BF),
        "be": np.ascontiguousarray(np.asarray(be, np.float32)[0]),
        "vst8": np.ascontiguousarray(
            np.asarray(Vs, np.float32).T).astype(mld.float8_e4m3),
        "bst": np.ascontiguousarray(np.asarray(bs, np.float32)[0]).astype(BF),
        "lt": LT.astype(BF),
        "lt2": (2.0 * LT).astype(BF),
        "wpb": wpb,
        "bias128": bias128,
    }


TRACE = False
LAST = {}


def kernel(x, edge_index, edge_weight, Ve, be, Vs, bs, cheb_W, cheb_b, res_W, res_b):
    from concourse.bass_utils import run_bass_kernel_spmd
    import ml_dtypes
    BF = ml_dtypes.bfloat16

    x = np.asarray(x, np.float32)
    shared = _host_prep(x, edge_index, edge_weight, Ve, be, Vs, bs,
                        cheb_W, cheb_b, res_W, res_b)
    nc = _build()
    in_maps = []
    for b in range(B):
        m = dict(shared)
        xb = x[b]                                   # (N, F, T)
        xnp = np.zeros((N, F, 32), np.float32)      # col 32f+t, zero padded
        xnp[:, :, :T] = xb
        m["xnp"] = np.ascontiguousarray(xnp.reshape(N, 1024)).astype(BF)
        xtf = xb.reshape(N, D).T                    # (768, N), d = f*24+t
        x96p = np.zeros((8, 4, 32, N), np.float32)  # row 128g+32j+u
        x96p[:, :, :T, :] = xtf.reshape(8, 4, T, N)
        m["xt96p"] = np.ascontiguousarray(x96p.reshape(8 * 128, N)).astype(BF)
        m["xtt"] = np.ascontiguousarray(
            xb.transpose(2, 1, 0).reshape(D, N)).astype(BF)  # d' = t*32+f
        in_maps.append(m)
    res = run_bass_kernel_spmd(nc, in_maps, list(range(B)), trace=TRACE)
    LAST["res"] = res
    out = np.stack(
        [r["out"].astype(np.float32).reshape(T, G, N).transpose(2, 1, 0)
         for r in res.results], axis=0)
    return out


# revision 31
# speedup vs baseline: 2.6427x; 1.0102x over previous
"""STBlock (temporal attn -> spatial attn -> ChebConv + residual, relu) on 8 trn2 cores.

Sharding: data-parallel over batch B=8, one batch element per core.

v2 design notes (vs 509us baseline): the baseline burned ~93us of PE on 256
tiny 24-col transposes and ~300us of Vector/Scalar on per-instruction copy
overhead. This version:
  - uploads x from host in all three layouts it is consumed in (natural
    f-major, transposed f-major, transposed t-major), killing the stage-3
    transpose storm entirely;
  - keeps a t-major (d' = t*32+f) column order for every intermediate, so the
    final Cheb+residual projection is 12 plain 128-contract matmuls with
    block weights and zero permutes/transposes at the tail;
  - computes each Chebyshev propagation directly in transposed form
    (Z1^T = sum_m Z0[m,d'] * L^T[m,n]), halving transpose passes;
  - applies E_att via a banded 768x768 block-diagonal matmul (15 tile pairs)
    built on-device from eatt with quadrant-safe small copies;
  - folds the bs-add into the S_pre PSUM accumulation via an identity matmul,
    and skips softmax max-subtraction (scores are provably < ~5);
  - batches every PSUM->SBUF drain to >=384-col single instructions and
    round-robins them across Vector/GpSimd/Scalar.

Layouts (per core):
  d  = f*24+t (f-major), d' = t*32+f (t-major); out^T row = t*64+g.
  All partition offsets must be 32-aligned (BIR verifier quadrant rule), so
  f-blocks (24 rows/cols) are placed at 32-strides with zero padding.
  XNP[8]  (128n, 1024)  bf16   natural padded (col 32f+t), for score_t
  XT96P[8](128d+,1024n) bf16   x^T f-major padded (row 32j+u per 4-f group)
  XTT[6]  (128d',1024n) bf16   x^T t-major, residual rhs in projection
  TT96[8] (96d, 1024n)  bf16   x_TA^T compact f-major (E-mult out)
  AN[8]   (128n, 768d') bf16   x_TA natural t-major (transpose+permute of TT)
  SG[8]   (128n, 1024m) bf16   sigmoid(score_s)
  SATB    (128m, 8x1024n) bf16 S_att^T, m-tile blocks along free dim
  Z0T/Z1T/Z2T[6] (128d', 1024n) bf16; Z0N/Z1N[8] (128n, 768d') bf16
  out^T   (1536, 1024) bf16 -> host transposes back
"""
import numpy as np

B, N, F, T, G = 8, 1024, 32, 24, 64
D = F * T            # 768
NCH = N // 128       # 8 n-chunks
DCH = D // 128       # 6 d-tiles
QO = 12              # out^T tiles (1536 rows)

_compiled = {}


def _build():
    if "nc" in _compiled:
        return _compiled["nc"]
    import concourse.mybir as mybir
    import concourse.bacc as bacc
    from concourse import tile

    FP = mybir.dt.float32
    BF = mybir.dt.bfloat16
    F8 = mybir.dt.float8e4
    AF = mybir.ActivationFunctionType
    OP = mybir.AluOpType
    DR = mybir.MatmulPerfMode.DoubleRow

    nc = bacc.Bacc("TRN2", target_bir_lowering=False, debug=False)

    xnp_d = nc.dram_tensor("xnp", (N, 1024), BF, kind="ExternalInput").ap()
    xt8f_d = nc.dram_tensor("xt8f", (D, N), F8, kind="ExternalInput").ap()
    bigi_d = nc.dram_tensor("bigi", (128, 384), BF, kind="ExternalInput").ap()
    xtt_d = nc.dram_tensor("xtt", (D, N), BF, kind="ExternalInput").ap()
    identb_d = nc.dram_tensor("identb", (128, 128), BF, kind="ExternalInput").ap()
    ident8_d = nc.dram_tensor("ident8", (128, 128), F8, kind="ExternalInput").ap()
    vetb_d = nc.dram_tensor("vetb", (T, T), BF, kind="ExternalInput").ap()
    be_d = nc.dram_tensor("be", (T, T), FP, kind="ExternalInput").ap()
    vst_d = nc.dram_tensor("vst8", (N, N), F8, kind="ExternalInput").ap()
    bst_d = nc.dram_tensor("bst", (N, N), BF, kind="ExternalInput").ap()
    lt_d = nc.dram_tensor("lt", (N, N), BF, kind="ExternalInput").ap()
    lt2_d = nc.dram_tensor("lt2", (N, N), BF, kind="ExternalInput").ap()
    wpb_d = nc.dram_tensor("wpb", (128, QO * 4 * 128), BF, kind="ExternalInput").ap()
    bias_d = nc.dram_tensor("bias128", (128, 1), FP, kind="ExternalInput").ap()
    out_d = nc.dram_tensor("out", (QO * 128, N), BF, kind="ExternalOutput").ap()

    with tile.TileContext(nc) as tc:
        with (
            tc.tile_pool(name="persist", bufs=1) as pp,
            tc.tile_pool(name="stream", bufs=1) as sp,
            tc.tile_pool(name="psb", bufs=2, space="PSUM") as psb,
            tc.tile_pool(name="pst", bufs=2, space="PSUM") as pst,
            tc.tile_pool(name="ps1", bufs=2, space="PSUM") as ps1,
        ):
            # round-robin for copy/cast work across DVE / Pool engines
            # (Act is kept for activations + a share of copies where idle)
            _rr = [0]
            PSUM_SPACE = tile.bass.MemorySpace.PSUM

            def copy_rr(dst, src, engines=None):
                if engines is None:
                    # GpSimd cannot touch PSUM
                    if src.space == PSUM_SPACE or dst.space == PSUM_SPACE:
                        engines = (nc.vector, nc.scalar)
                    else:
                        engines = (nc.vector, nc.gpsimd)
                e = engines[_rr[0] % len(engines)]
                _rr[0] += 1
                if e is nc.scalar:
                    nc.scalar.activation(dst, src, AF.Copy)
                else:
                    e.tensor_copy(dst, src)

            # ---- constants / inputs ----
            identb = pp.tile([128, 128], BF, tag="identb")
            nc.sync.dma_start(identb[:], identb_d[:])
            ident8 = pp.tile([128, 128], F8, tag="ident8")
            nc.sync.dma_start(ident8[:], ident8_d[:])
            vetb = pp.tile([T, T], BF, tag="vetb")
            nc.sync.dma_start(vetb[:], vetb_d[:])
            be = pp.tile([T, T], FP, tag="be")
            nc.sync.dma_start(be[:], be_d[:])
            # preload Act function tables off the critical path
            warm = sp.tile([1, 1], FP, tag="warm")
            nc.scalar.activation(warm[:], identb[0:1, 0:1], AF.Sigmoid)
            nc.scalar.activation(warm[:], identb[0:1, 0:1], AF.Exp)
            nc.scalar.activation(warm[:], identb[0:1, 0:1], AF.Relu)
            wpb = pp.tile([128, QO * 4 * 128], BF, tag="wpb")
            nc.sync.dma_start(wpb[:], wpb_d[:])
            bias128 = pp.tile([128, 1], FP, tag="bias128")
            nc.sync.dma_start(bias128[:], bias_d[:])

            XNP = []
            for i in range(NCH):
                t_ = pp.tile([128, 1024], BF, name=f"xnpA{i}", tag=f"A{i}")
                nc.sync.dma_start(t_[:], xnp_d[i * 128:(i + 1) * 128, :])
                XNP.append(t_)
            # x^T f-major fp8, one tile: col block p = d-tile p (DR pairing)
            xt8f = pp.tile([128, DCH * N], F8, tag="xt8f")
            for p in range(DCH):
                nc.sync.dma_start(xt8f[:, p * N:(p + 1) * N],
                                  xt8f_d[p * 128:(p + 1) * 128, :])
            xt8fv = xt8f[:].rearrange("q (p n) -> q p n", p=DCH)
            bigi = pp.tile([128, 384], BF, tag="bigi")
            nc.sync.dma_start(bigi[:], bigi_d[:])
            XTT = []
            for p in range(DCH):
                t_ = pp.tile([128, N], BF, name=f"xttX{p}", tag=f"X{p}")
                nc.sync.dma_start(t_[:], xtt_d[p * 128:(p + 1) * 128, :])
                XTT.append(t_)
            # Vs^T as one (128, 8x1024) fp8 tile: col block m holds m-tile rows
            vst8 = pp.tile([128, NCH * N], F8, tag="vst8")
            for m in range(NCH):
                nc.sync.dma_start(vst8[:, m * N:(m + 1) * N],
                                  vst_d[m * 128:(m + 1) * 128, :])
            vst8v = vst8[:].rearrange("q (m n) -> q m n", m=NCH)

            # ---- S1: score_t = sum_{n,f} x[n,f,t] x[n,f,u] ----
            # XNP col blocks of 128 = 4 f's at 32-stride padding; the Gram of
            # each block has the per-f 24x24 diagonal blocks at 32-aligned
            # partition offsets. Garbage off-diagonal blocks are ignored.
            acc128 = pp.tile([128, 128], FP, tag="acc128")
            for g2 in range(8):
                pt = ps1.tile([128, 128], FP, tag="st")
                for i in range(NCH):
                    sl = XNP[i][:, g2 * 128:(g2 + 1) * 128]
                    nc.tensor.matmul(pt[:], sl, sl,
                                     start=(i == 0), stop=(i == NCH - 1))
                if g2 == 0:
                    nc.vector.tensor_copy(acc128[:], pt[:])
                else:
                    nc.vector.tensor_tensor(acc128[:], acc128[:], pt[:], op=OP.add)
            # TensorTensor needs equal base partitions for SBUF inputs, so
            # first move the three off-base diagonal blocks down to base 0.
            dg = []
            for j, eng in ((1, nc.vector), (2, nc.gpsimd), (3, nc.vector)):
                t_ = sp.tile([T, T], FP, name=f"dg{j}", tag=f"dg{j}")
                eng.tensor_copy(t_[:], acc128[32 * j:32 * j + 24,
                                              32 * j:32 * j + 24])
                dg.append(t_)
            sct_a = sp.tile([T, T], FP, tag="sct_a")
            nc.vector.tensor_tensor(sct_a[:], acc128[0:24, 0:24],
                                    dg[0][:], op=OP.add)
            sct_b = sp.tile([T, T], FP, tag="sct_b")
            nc.gpsimd.tensor_tensor(sct_b[:], dg[1][:], dg[2][:], op=OP.add)
            score_t = sp.tile([T, T], FP, tag="score_t")
            nc.vector.tensor_tensor(score_t[:], sct_a[:], sct_b[:], op=OP.add)

            # ---- S2: E_att = softmax(Ve @ sigmoid(score_t) + be) ----
            sigb = sp.tile([T, T], BF, tag="sigb")
            nc.scalar.activation(sigb[:], score_t[:], AF.Sigmoid)
            ps_e = ps1.tile([T, T], FP, tag="st")
            nc.tensor.matmul(ps_e[:], vetb[:], sigb[:], start=True, stop=True)
            epre = sp.tile([T, T], FP, tag="epre")
            nc.vector.tensor_tensor(epre[:], ps_e[:], be[:], op=OP.add)
            eexp = sp.tile([T, T], FP, tag="eexp")
            esum = sp.tile([T, 1], FP, tag="esum")
            nc.scalar.activation(eexp[:], epre[:], AF.Exp, accum_out=esum[:])
            einv = sp.tile([T, 1], FP, tag="einv")
            nc.vector.reciprocal(einv[:], esum[:])
            eatt = sp.tile([T, T], BF, tag="eatt")
            nc.vector.tensor_scalar_mul(eatt[:], eexp[:], einv[:])

            # EBIG: banded blocks of blockdiag(E_att x32), built on the PE
            # with shift-matrix (identity-slice) matmuls, then cast to fp8.
            bands = []
            for p in range(DCH):
                qs = []
                for q in (p - 1, p, p + 1):
                    if not 0 <= q < DCH:
                        continue
                    fs = [f for f in range(F)
                          if 24 * f < 128 * q + 128 and 24 * f + 24 > 128 * q
                          and 24 * f < 128 * p + 128 and 24 * f + 24 > 128 * p]
                    if fs:
                        qs.append((q, fs))
                bands.append(qs)
            soff = {}
            s = 0
            for p in range(DCH):
                for q, _ in bands[p]:
                    soff[(p, q)] = s
                    s += 1
            NB = s  # 14 blocks
            e4r = pp.tile([128, T], BF, tag="e4r")
            nc.gpsimd.memset(e4r[:], 0.0)
            nc.vector.tensor_copy(e4r[0:24, :], eatt[:])
            ebig = pp.tile([128, NB * 128], F8, tag="ebig")
            nc.gpsimd.memset(ebig[:], 0.0)
            for half in range(2):
                blo = half * 7
                bhi = min(NB, blo + 7)
                pe_b = psb.tile([128, N], FP, tag="big")
                ranges = {}
                for p in range(DCH):
                    for q, fs in bands[p]:
                        sb = soff[(p, q)]
                        if not blo <= sb < bhi:
                            continue
                        for f in fs:
                            dlt = 24 * f - 128 * q
                            c0 = 24 * f - 128 * p
                            t0, t1 = max(0, -c0), min(24, 128 - c0)
                            cc = (sb - blo) * 128 + c0 + t0
                            nc.tensor.matmul(
                                pe_b[:, cc:cc + (t1 - t0)],
                                bigi[:, 128 - dlt:256 - dlt],
                                e4r[:, t0:t1], start=True, stop=True)
                            lo, hi = ranges.get(sb, (10 ** 9, -1))
                            ranges[sb] = (min(lo, c0 + t0), max(hi, c0 + t1))
                for sb, (lo, hi) in sorted(ranges.items()):
                    copy_rr(ebig[:, sb * 128 + lo:sb * 128 + hi],
                            pe_b[:, (sb - blo) * 128 + lo:(sb - blo) * 128 + hi])

            # ---- S3: TT8 = x_TA^T (f-major) via banded fp8 matmul ----
            tt8 = pp.tile([128, DCH * N], F8, tag="tt8")
            for p in range(DCH):
                pb = psb.tile([128, N], FP, tag="big")
                qs = bands[p]
                q0 = qs[0][0]
                s0 = soff[(p, q0)]
                for h in range(2):
                    nc.tensor.matmul(
                        pb[:, h * 512:(h + 1) * 512],
                        ebig[:, s0 * 128:(s0 + 2) * 128].rearrange(
                            "q (k c) -> q k c", k=2),
                        xt8fv[:, q0:q0 + 2, h * 512:(h + 1) * 512],
                        start=True, stop=(len(qs) == 2), perf_mode=DR)
                    if len(qs) == 3:
                        q2 = qs[2][0]
                        s2 = soff[(p, q2)]
                        nc.tensor.matmul(
                            pb[:, h * 512:(h + 1) * 512],
                            ebig[:, s2 * 128:(s2 + 1) * 128],
                            xt8f[:, q2 * N + h * 512:q2 * N + (h + 1) * 512],
                            start=False, stop=True)
                copy_rr(tt8[:, p * N:(p + 1) * N], pb[:])
            tt8v = tt8[:].rearrange("q (p n) -> q p n", p=DCH)

            # ---- S5 (score_s -> SG) interleaved with S4 (AN build) ----
            sg8 = pp.tile([128, NCH * N], F8, tag="sg8")
            sg8v = sg8[:].rearrange("q (m n) -> q m n", m=NCH)
            anb = pp.tile([128, NCH * D], F8, tag="anb")
            anbv = anb[:].rearrange("q (m d) -> q m d", m=NCH)
            for i in range(NCH):
                pb = psb.tile([128, N], FP, tag="big")
                for h in range(2):
                    for a2 in range(3):  # DoubleRow over d-tile pairs
                        nc.tensor.matmul(
                            pb[:, h * 512:(h + 1) * 512],
                            tt8v[:, 2 * a2:2 * a2 + 2, i * 128:(i + 1) * 128],
                            tt8v[:, 2 * a2:2 * a2 + 2, h * 512:(h + 1) * 512],
                            start=(a2 == 0), stop=(a2 == 2), perf_mode=DR)
                nc.scalar.activation(sg8[:, i * N:(i + 1) * N], pb[:], AF.Sigmoid)

                # fp8 transpose must write psum with element step 2
                pa = pst.tile([128, 2 * D], F8, tag="tr")
                pav = pa[:].rearrange("q (c two) -> q two c", two=2)
                for p in range(DCH):
                    nc.tensor.transpose(pav[:, 0, p * 128:(p + 1) * 128],
                                        tt8[:, p * N + i * 128:p * N + (i + 1) * 128],
                                        ident8[:])
                # one strided copy: drop the step-2 padding and permute
                # f-major d -> t-major d' in the same instruction
                copy_rr(anb[:, i * D:(i + 1) * D].rearrange(
                            "q (t f) -> q f t", t=T, f=F).unsqueeze(3),
                        pa[:].rearrange("q (f t two) -> q f t two",
                                        f=F, t=T, two=2)[:, :, :, 0:1])

            # ---- S6: S_att rows + softmax; S7: transpose into SATB ----
            satb = pp.tile([128, NCH * N], F8, tag="satb")
            satbv = satb[:].rearrange("q (m n) -> q m n", m=NCH)
            for i in range(NCH):
                bsb = sp.tile([128, N], BF, tag="bsb", bufs=2)
                nc.sync.dma_start(bsb[:], bst_d[i * 128:(i + 1) * 128, :])
                pb = psb.tile([128, N], FP, tag="big")
                for h in range(2):
                    for a2 in range(4):  # DoubleRow over m-tile pairs
                        nc.tensor.matmul(
                            pb[:, h * 512:(h + 1) * 512],
                            vst8v[:, 2 * a2:2 * a2 + 2, i * 128:(i + 1) * 128],
                            sg8v[:, 2 * a2:2 * a2 + 2, h * 512:(h + 1) * 512],
                            start=(a2 == 0), stop=False, perf_mode=DR)
                    nc.tensor.matmul(
                        pb[:, h * 512:(h + 1) * 512],
                        identb[:], bsb[:, h * 512:(h + 1) * 512],
                        start=False, stop=True)
                sexp = sp.tile([128, N], FP, tag="sexp", bufs=2)
                ssum = sp.tile([128, 1], FP, tag="ssum", bufs=2)
                nc.scalar.activation(sexp[:], pb[:], AF.Exp, accum_out=ssum[:])
                sinv = sp.tile([128, 1], FP, tag="sinv", bufs=2)
                nc.vector.reciprocal(sinv[:], ssum[:])
                # x512 keeps softmax weights above the fp8e4m3 subnormal floor;
                # the Z0T drain divides it back out.
                sa = sp.tile([128, N], BF, tag="sa", bufs=2)
                nc.vector.tensor_scalar(sa[:], sexp[:], sinv[:], 512.0,
                                        op0=OP.mult, op1=OP.mult)
                for grp in range(2):
                    pq = pst.tile([128, 512], BF, tag="tr")
                    for k in range(4):
                        p = grp * 4 + k
                        nc.tensor.transpose(pq[:, k * 128:(k + 1) * 128],
                                            sa[:, p * 128:(p + 1) * 128],
                                            identb[:])
                    dst = satb[:].rearrange("q (p n) -> q p n", p=NCH)[
                        :, grp * 4:grp * 4 + 4, i * 128:(i + 1) * 128]
                    copy_rr(dst, pq[:].rearrange("q (p n) -> q p n", p=4))

            # LT into SG slots (SG dead), LT2 into B slots + 2 fresh
            LT = []
            for m in range(NCH):
                t_ = pp.tile([128, N], BF, name=f"ltS{m}", tag=f"S{m}")
                nc.sync.dma_start(t_[:], lt_d[m * 128:(m + 1) * 128, :])
                LT.append(t_)
            LT2 = []
            for m in range(NCH):
                t_ = pp.tile([128, N], BF, name=f"lt2{m}", tag=f"B{m}")
                nc.sync.dma_start(t_[:], lt2_d[m * 128:(m + 1) * 128, :])
                LT2.append(t_)

            # ---- S8: Z0T = (S_att @ x_TA)^T directly (t-major) ----
            Z0T = []
            for p in range(DCH):
                pb = psb.tile([128, N], FP, tag="big")
                for h in range(2):
                    for a2 in range(4):  # DoubleRow over m-tile pairs
                        nc.tensor.matmul(
                            pb[:, h * 512:(h + 1) * 512],
                            anbv[:, 2 * a2:2 * a2 + 2, p * 128:(p + 1) * 128],
                            satbv[:, 2 * a2:2 * a2 + 2, h * 512:(h + 1) * 512],
                            start=(a2 == 0), stop=(a2 == 3), perf_mode=DR)
                t_ = pp.tile([128, N], BF, name=f"z0tT{p}", tag=f"T{p}")
                if p % 2 == 0:
                    nc.vector.tensor_scalar_mul(t_[:], pb[:], 1.0 / 512.0)
                else:
                    nc.scalar.activation(t_[:], pb[:], AF.Copy, scale=1.0 / 512.0)
                Z0T.append(t_)

            # ---- S9: Z0N = transpose(Z0T) ----
            Z0N = []
            for i in range(NCH):
                z = pp.tile([128, D], BF, name=f"z0nA{i}", tag=f"A{i}")
                Z0N.append(z)
            for i in range(NCH):
                for grp in range(2):
                    pz = pst.tile([128, 384], BF, tag="tr")
                    for k in range(3):
                        p = grp * 3 + k
                        nc.tensor.transpose(pz[:, k * 128:(k + 1) * 128],
                                            Z0T[p][:, i * 128:(i + 1) * 128],
                                            identb[:])
                    copy_rr(Z0N[i][:, grp * 384:(grp + 1) * 384], pz[:])

            # ---- S10: Z1T[d',n] = sum_m Z0[m,d'] L^T[m,n] = (L@Z0)^T ----
            Z1T = []
            for p in range(DCH):
                pb = psb.tile([128, N], FP, tag="big")
                for h in range(2):
                    for m in range(NCH):
                        nc.tensor.matmul(
                            pb[:, h * 512:(h + 1) * 512],
                            Z0N[m][:, p * 128:(p + 1) * 128],
                            LT[m][:, h * 512:(h + 1) * 512],
                            start=(m == 0), stop=(m == NCH - 1))
                t_ = pp.tile([128, N], BF, name=f"z1tV{p}", tag=f"V{p}")
                copy_rr(t_[:], pb[:])
                Z1T.append(t_)

            # ---- S11: Z1N = transpose(Z1T) ----
            Z1N = []
            for i in range(NCH):
                z = pp.tile([128, D], BF, name=f"z1nN{i}", tag=f"AN{i}")
                Z1N.append(z)
            for i in range(NCH):
                for grp in range(2):
                    pz = pst.tile([128, 384], BF, tag="tr")
                    for k in range(3):
                        p = grp * 3 + k
                        nc.tensor.transpose(pz[:, k * 128:(k + 1) * 128],
                                            Z1T[p][:, i * 128:(i + 1) * 128],
                                            identb[:])
                    copy_rr(Z1N[i][:, grp * 384:(grp + 1) * 384], pz[:])

            # ---- S12: Z2T = 2*(L@Z1)^T - Z0T ----
            Z2T = []
            for p in range(DCH):
                pb = psb.tile([128, N], FP, tag="big")
                for h in range(2):
                    for m in range(NCH):
                        nc.tensor.matmul(
                            pb[:, h * 512:(h + 1) * 512],
                            Z1N[m][:, p * 128:(p + 1) * 128],
                            LT2[m][:, h * 512:(h + 1) * 512],
                            start=(m == 0), stop=(m == NCH - 1))
                t_ = pp.tile([128, N], BF, name=f"z2tZ{p}", tag=f"Z2{p}")
                if p % 2 == 0:
                    nc.vector.tensor_tensor(t_[:], pb[:], Z0T[p][:], op=OP.subtract)
                else:
                    # spread load: Act drains PSUM, Pool does the SBUF subtract
                    pc = sp.tile([128, N], BF, tag="z2c", bufs=2)
                    nc.scalar.activation(pc[:], pb[:], AF.Copy)
                    nc.gpsimd.tensor_tensor(t_[:], pc[:], Z0T[p][:], op=OP.subtract)
                Z2T.append(t_)

            # ---- S13: projection (Cheb k=0..2 + residual), bias, relu ----
            for q in range(QO):
                p = q // 2
                pb = psb.tile([128, N], FP, tag="big")
                rhs4 = (Z0T[p], Z1T[p], Z2T[p], XTT[p])
                for h in range(2):
                    for k in range(4):
                        nc.tensor.matmul(
                            pb[:, h * 512:(h + 1) * 512],
                            wpb[:, (4 * q + k) * 128:(4 * q + k + 1) * 128],
                            rhs4[k][:, h * 512:(h + 1) * 512],
                            start=(k == 0), stop=(k == 3))
                ob = sp.tile([128, N], BF, tag="outbuf", bufs=2)
                nc.scalar.activation(ob[:], pb[:], AF.Relu, bias=bias128[:])
                nc.sync.dma_start(out_d[q * 128:(q + 1) * 128, :], ob[:])

    nc.compile()
    _compiled["nc"] = nc
    return nc


def _host_prep(x, edge_index, edge_weight, Ve, be, Vs, bs, cheb_W, cheb_b, res_W, res_b):
    import ml_dtypes
    BF = ml_dtypes.bfloat16
    row = np.asarray(edge_index[0]).astype(np.int64)
    col = np.asarray(edge_index[1]).astype(np.int64)
    w = np.asarray(edge_weight, np.float64).copy()
    w[row == col] = 0.0
    deg = np.zeros(N, np.float64)
    np.add.at(deg, row, w)
    dis = np.where(deg > 0, 1.0 / np.sqrt(np.where(deg > 0, deg, 1.0)), 0.0)
    norm = -dis[row] * w * dis[col]
    L = np.zeros((N, N), np.float64)
    np.add.at(L, (col, row), norm)
    LT = np.ascontiguousarray(L.T.astype(np.float32))

    cheb_W = np.asarray(cheb_W, np.float32)
    res_W = np.asarray(res_W, np.float32)
    # wpb[p, (4q+k)*128 + c] = blk(q,k)[p, c]; out^T tile q rows (t,g) with
    # t = 2q + c//64, contracting t-major tile p=q//2 rows (t', f)
    wq = np.zeros((QO, 4, 128, 128), np.float32)
    Wlist = [cheb_W[0], cheb_W[1], cheb_W[2], res_W.T]  # each (F, G)
    for q in range(QO):
        off = 0 if q % 2 == 0 else 2
        for b_ in range(2):
            a = b_ + off
            for k in range(4):
                wq[q, k, 32 * a:32 * a + 32, 64 * b_:64 * b_ + 64] = Wlist[k]
    wpb = np.ascontiguousarray(
        wq.transpose(2, 0, 1, 3).reshape(128, QO * 4 * 128)).astype(BF)

    b64 = (np.asarray(cheb_b, np.float32) + np.asarray(res_b, np.float32))
    bias128 = np.concatenate([b64, b64]).reshape(128, 1).astype(np.float32)

    import ml_dtypes as mld
    bigi = np.zeros((128, 384), np.float32)
    bigi[np.arange(128), 128 + np.arange(128)] = 1.0
    return {
        "bigi": bigi.astype(mld.bfloat16),
        "identb": np.eye(128, dtype=np.float32).astype(BF),
        "ident8": np.eye(128, dtype=np.float32).astype(mld.float8_e4m3),
        "vetb": np.ascontiguousarray(np.asarray(Ve, np.float32).T).astype(BF),
        "be": np.ascontiguousarray(np.asarray(be, np.float32)[0]),
        "vst8": np.ascontiguousarray(
            np.asarray(Vs, np.float32).T).astype(mld.float8_e4m3),
        "bst": np.ascontiguousarray(np.asarray(bs, np.float32)[0]).astype(BF),
        "lt": LT.astype(BF),
        "lt2": (2.0 * LT).astype(BF),
        "wpb": wpb,
        "bias128": bias128,
    }


TRACE = False
LAST = {}


def kernel(x, edge_index, edge_weight, Ve, be, Vs, bs, cheb_W, cheb_b, res_W, res_b):
    from concourse.bass_utils import run_bass_kernel_spmd
    import ml_dtypes
    BF = ml_dtypes.bfloat16
    F8H = ml_dtypes.float8_e4m3

    x = np.asarray(x, np.float32)
    shared = _host_prep(x, edge_index, edge_weight, Ve, be, Vs, bs,
                        cheb_W, cheb_b, res_W, res_b)
    nc = _build()
    in_maps = []
    for b in range(B):
        m = dict(shared)
        xb = x[b]                                   # (N, F, T)
        xnp = np.zeros((N, F, 32), np.float32)      # col 32f+t, zero padded
        xnp[:, :, :T] = xb
        m["xnp"] = np.ascontiguousarray(xnp.reshape(N, 1024)).astype(BF)
        xtf = xb.reshape(N, D).T                    # (768, N), d = f*24+t
        m["xt8f"] = np.ascontiguousarray(xtf).astype(F8H)
        m["xtt"] = np.ascontiguousarray(
            xb.transpose(2, 1, 0).reshape(D, N)).astype(BF)  # d' = t*32+f
        in_maps.append(m)
    res = run_bass_kernel_spmd(nc, in_maps, list(range(B)), trace=TRACE)
    LAST["res"] = res
    out = np.stack(
        [r["out"].astype(np.float32).reshape(T, G, N).transpose(2, 1, 0)
         for r in res.results], axis=0)
    return out


# revision 33
# speedup vs baseline: 2.9605x; 1.1203x over previous
"""STBlock (temporal attn -> spatial attn -> ChebConv + residual, relu) on 8 trn2 cores.

Sharding: data-parallel over batch B=8, one batch element per core.

v2 design notes (vs 509us baseline): the baseline burned ~93us of PE on 256
tiny 24-col transposes and ~300us of Vector/Scalar on per-instruction copy
overhead. This version:
  - uploads x from host in all three layouts it is consumed in (natural
    f-major, transposed f-major, transposed t-major), killing the stage-3
    transpose storm entirely;
  - keeps a t-major (d' = t*32+f) column order for every intermediate, so the
    final Cheb+residual projection is 12 plain 128-contract matmuls with
    block weights and zero permutes/transposes at the tail;
  - computes each Chebyshev propagation directly in transposed form
    (Z1^T = sum_m Z0[m,d'] * L^T[m,n]), halving transpose passes;
  - applies E_att via a banded 768x768 block-diagonal matmul (15 tile pairs)
    built on-device from eatt with quadrant-safe small copies;
  - folds the bs-add into the S_pre PSUM accumulation via an identity matmul,
    and skips softmax max-subtraction (scores are provably < ~5);
  - batches every PSUM->SBUF drain to >=384-col single instructions and
    round-robins them across Vector/GpSimd/Scalar.

Layouts (per core):
  d  = f*24+t (f-major), d' = t*32+f (t-major); out^T row = t*64+g.
  All partition offsets must be 32-aligned (BIR verifier quadrant rule), so
  f-blocks (24 rows/cols) are placed at 32-strides with zero padding.
  XNP[8]  (128n, 1024)  bf16   natural padded (col 32f+t), for score_t
  XT96P[8](128d+,1024n) bf16   x^T f-major padded (row 32j+u per 4-f group)
  XTT[6]  (128d',1024n) bf16   x^T t-major, residual rhs in projection
  TT96[8] (96d, 1024n)  bf16   x_TA^T compact f-major (E-mult out)
  AN[8]   (128n, 768d') bf16   x_TA natural t-major (transpose+permute of TT)
  SG[8]   (128n, 1024m) bf16   sigmoid(score_s)
  SATB    (128m, 8x1024n) bf16 S_att^T, m-tile blocks along free dim
  Z0T/Z1T/Z2T[6] (128d', 1024n) bf16; Z0N/Z1N[8] (128n, 768d') bf16
  out^T   (1536, 1024) bf16 -> host transposes back
"""
import numpy as np

B, N, F, T, G = 8, 1024, 32, 24, 64
D = F * T            # 768
NCH = N // 128       # 8 n-chunks
DCH = D // 128       # 6 d-tiles
QO = 12              # out^T tiles (1536 rows)

_compiled = {}


def _build():
    if "nc" in _compiled:
        return _compiled["nc"]
    import concourse.mybir as mybir
    import concourse.bacc as bacc
    from concourse import tile

    FP = mybir.dt.float32
    BF = mybir.dt.bfloat16
    F8 = mybir.dt.float8e4
    AF = mybir.ActivationFunctionType
    OP = mybir.AluOpType
    DR = mybir.MatmulPerfMode.DoubleRow

    nc = bacc.Bacc("TRN2", target_bir_lowering=False, debug=False)

    xnp_d = nc.dram_tensor("xnp", (N, 1024), BF, kind="ExternalInput").ap()
    xt8f_d = nc.dram_tensor("xt8f", (D, N), F8, kind="ExternalInput").ap()
    bigi_d = nc.dram_tensor("bigi", (128, 384), BF, kind="ExternalInput").ap()
    xtt_d = nc.dram_tensor("xtt", (D, N), BF, kind="ExternalInput").ap()
    identb_d = nc.dram_tensor("identb", (128, 128), BF, kind="ExternalInput").ap()
    ident8_d = nc.dram_tensor("ident8", (128, 128), F8, kind="ExternalInput").ap()
    vetb_d = nc.dram_tensor("vetb", (T, T), BF, kind="ExternalInput").ap()
    be_d = nc.dram_tensor("be", (T, T), FP, kind="ExternalInput").ap()
    vst_d = nc.dram_tensor("vst8", (N, N), F8, kind="ExternalInput").ap()
    bst_d = nc.dram_tensor("bst", (N, N), BF, kind="ExternalInput").ap()
    lt8_d = nc.dram_tensor("lt8", (N, N), F8, kind="ExternalInput").ap()
    wpb_d = nc.dram_tensor("wpb", (128, QO * 4 * 128), BF, kind="ExternalInput").ap()
    bias_d = nc.dram_tensor("bias128", (128, 1), FP, kind="ExternalInput").ap()
    out_d = nc.dram_tensor("out", (QO * 128, N), BF, kind="ExternalOutput").ap()

    with tile.TileContext(nc) as tc:
        with (
            tc.tile_pool(name="persist", bufs=1) as pp,
            tc.tile_pool(name="stream", bufs=1) as sp,
            tc.tile_pool(name="psb", bufs=2, space="PSUM") as psb,
            tc.tile_pool(name="pst", bufs=2, space="PSUM") as pst,
            tc.tile_pool(name="ps1", bufs=2, space="PSUM") as ps1,
        ):
            # round-robin for copy/cast work across DVE / Pool engines
            # (Act is kept for activations + a share of copies where idle)
            _rr = [0]
            PSUM_SPACE = tile.bass.MemorySpace.PSUM

            def copy_rr(dst, src, engines=None):
                if engines is None:
                    # GpSimd cannot touch PSUM
                    if src.space == PSUM_SPACE or dst.space == PSUM_SPACE:
                        engines = (nc.vector, nc.scalar)
                    else:
                        engines = (nc.vector, nc.gpsimd)
                e = engines[_rr[0] % len(engines)]
                _rr[0] += 1
                if e is nc.scalar:
                    nc.scalar.activation(dst, src, AF.Copy)
                else:
                    e.tensor_copy(dst, src)

            def scaled_rr(dst, src, scale):
                if _rr[0] % 2 == 0:
                    nc.vector.tensor_scalar_mul(dst, src, scale)
                else:
                    nc.scalar.activation(dst, src, AF.Copy, scale=scale)
                _rr[0] += 1

            # ---- constants / inputs ----
            identb = pp.tile([128, 128], BF, tag="identb")
            nc.sync.dma_start(identb[:], identb_d[:])
            ident8 = pp.tile([128, 128], F8, tag="ident8")
            nc.sync.dma_start(ident8[:], ident8_d[:])
            vetb = pp.tile([T, T], BF, tag="vetb")
            nc.sync.dma_start(vetb[:], vetb_d[:])
            be = pp.tile([T, T], FP, tag="be")
            nc.sync.dma_start(be[:], be_d[:])
            # preload Act function tables off the critical path
            warm = sp.tile([1, 1], FP, tag="warm")
            nc.scalar.activation(warm[:], identb[0:1, 0:1], AF.Sigmoid)
            nc.scalar.activation(warm[:], identb[0:1, 0:1], AF.Exp)
            nc.scalar.activation(warm[:], identb[0:1, 0:1], AF.Relu)
            wpb = pp.tile([128, QO * 4 * 128], BF, tag="wpb")
            nc.sync.dma_start(wpb[:], wpb_d[:])
            bias128 = pp.tile([128, 1], FP, tag="bias128")
            nc.sync.dma_start(bias128[:], bias_d[:])

            XNP = []
            for i in range(NCH):
                t_ = pp.tile([128, 1024], BF, name=f"xnpA{i}", tag=f"A{i}")
                nc.sync.dma_start(t_[:], xnp_d[i * 128:(i + 1) * 128, :])
                XNP.append(t_)
            # x^T f-major fp8, one tile: col block p = d-tile p (DR pairing)
            xt8f = pp.tile([128, DCH * N], F8, tag="xt8f")
            for p in range(DCH):
                nc.sync.dma_start(xt8f[:, p * N:(p + 1) * N],
                                  xt8f_d[p * 128:(p + 1) * 128, :])
            xt8fv = xt8f[:].rearrange("q (p n) -> q p n", p=DCH)
            bigi = pp.tile([128, 384], BF, tag="bigi")
            nc.sync.dma_start(bigi[:], bigi_d[:])
            XTT = []
            for p in range(DCH):
                t_ = pp.tile([128, N], BF, name=f"xttX{p}", tag=f"X{p}")
                nc.sync.dma_start(t_[:], xtt_d[p * 128:(p + 1) * 128, :])
                XTT.append(t_)
            # Vs^T as one (128, 8x1024) fp8 tile: col block m holds m-tile rows
            vst8 = pp.tile([128, NCH * N], F8, tag="vst8")
            for m in range(NCH):
                nc.sync.dma_start(vst8[:, m * N:(m + 1) * N],
                                  vst_d[m * 128:(m + 1) * 128, :])
            vst8v = vst8[:].rearrange("q (m n) -> q m n", m=NCH)

            # ---- S1: score_t = sum_{n,f} x[n,f,t] x[n,f,u] ----
            # XNP col blocks of 128 = 4 f's at 32-stride padding; the Gram of
            # each block has the per-f 24x24 diagonal blocks at 32-aligned
            # partition offsets. Garbage off-diagonal blocks are ignored.
            acc128 = pp.tile([128, 128], FP, tag="acc128")
            for g2 in range(8):
                pt = ps1.tile([128, 128], FP, tag="st")
                for i in range(NCH):
                    sl = XNP[i][:, g2 * 128:(g2 + 1) * 128]
                    nc.tensor.matmul(pt[:], sl, sl,
                                     start=(i == 0), stop=(i == NCH - 1))
                if g2 == 0:
                    nc.vector.tensor_copy(acc128[:], pt[:])
                else:
                    nc.vector.tensor_tensor(acc128[:], acc128[:], pt[:], op=OP.add)
            # TensorTensor needs equal base partitions for SBUF inputs, so
            # first move the three off-base diagonal blocks down to base 0.
            dg = []
            for j, eng in ((1, nc.vector), (2, nc.gpsimd), (3, nc.vector)):
                t_ = sp.tile([T, T], FP, name=f"dg{j}", tag=f"dg{j}")
                eng.tensor_copy(t_[:], acc128[32 * j:32 * j + 24,
                                              32 * j:32 * j + 24])
                dg.append(t_)
            sct_a = sp.tile([T, T], FP, tag="sct_a")
            nc.vector.tensor_tensor(sct_a[:], acc128[0:24, 0:24],
                                    dg[0][:], op=OP.add)
            sct_b = sp.tile([T, T], FP, tag="sct_b")
            nc.gpsimd.tensor_tensor(sct_b[:], dg[1][:], dg[2][:], op=OP.add)
            score_t = sp.tile([T, T], FP, tag="score_t")
            nc.vector.tensor_tensor(score_t[:], sct_a[:], sct_b[:], op=OP.add)

            # ---- S2: E_att = softmax(Ve @ sigmoid(score_t) + be) ----
            sigb = sp.tile([T, T], BF, tag="sigb")
            nc.scalar.activation(sigb[:], score_t[:], AF.Sigmoid)
            ps_e = ps1.tile([T, T], FP, tag="st")
            nc.tensor.matmul(ps_e[:], vetb[:], sigb[:], start=True, stop=True)
            epre = sp.tile([T, T], FP, tag="epre")
            nc.vector.tensor_tensor(epre[:], ps_e[:], be[:], op=OP.add)
            eexp = sp.tile([T, T], FP, tag="eexp")
            esum = sp.tile([T, 1], FP, tag="esum")
            nc.scalar.activation(eexp[:], epre[:], AF.Exp, accum_out=esum[:])
            einv = sp.tile([T, 1], FP, tag="einv")
            nc.vector.reciprocal(einv[:], esum[:])
            eatt = sp.tile([T, T], BF, tag="eatt")
            nc.vector.tensor_scalar_mul(eatt[:], eexp[:], einv[:])

            # EBIG: banded blocks of blockdiag(E_att x32), built on the PE
            # with shift-matrix (identity-slice) matmuls, then cast to fp8.
            bands = []
            for p in range(DCH):
                qs = []
                for q in (p - 1, p, p + 1):
                    if not 0 <= q < DCH:
                        continue
                    fs = [f for f in range(F)
                          if 24 * f < 128 * q + 128 and 24 * f + 24 > 128 * q
                          and 24 * f < 128 * p + 128 and 24 * f + 24 > 128 * p]
                    if fs:
                        qs.append((q, fs))
                bands.append(qs)
            soff = {}
            s = 0
            for p in range(DCH):
                for q, _ in bands[p]:
                    soff[(p, q)] = s
                    s += 1
            NB = s  # 14 blocks
            e4r = pp.tile([128, T], BF, tag="e4r")
            nc.gpsimd.memset(e4r[:], 0.0)
            nc.vector.tensor_copy(e4r[0:24, :], eatt[:])
            ebig = pp.tile([128, NB * 128], F8, tag="ebig")
            nc.gpsimd.memset(ebig[:], 0.0)
            for half in range(2):
                blo = half * 7
                bhi = min(NB, blo + 7)
                pe_b = psb.tile([128, N], FP, tag="big")
                ranges = {}
                for p in range(DCH):
                    for q, fs in bands[p]:
                        sb = soff[(p, q)]
                        if not blo <= sb < bhi:
                            continue
                        for f in fs:
                            dlt = 24 * f - 128 * q
                            c0 = 24 * f - 128 * p
                            t0, t1 = max(0, -c0), min(24, 128 - c0)
                            cc = (sb - blo) * 128 + c0 + t0
                            nc.tensor.matmul(
                                pe_b[:, cc:cc + (t1 - t0)],
                                bigi[:, 128 - dlt:256 - dlt],
                                e4r[:, t0:t1], start=True, stop=True)
                            lo, hi = ranges.get(sb, (10 ** 9, -1))
                            ranges[sb] = (min(lo, c0 + t0), max(hi, c0 + t1))
                for sb, (lo, hi) in sorted(ranges.items()):
                    copy_rr(ebig[:, sb * 128 + lo:sb * 128 + hi],
                            pe_b[:, (sb - blo) * 128 + lo:(sb - blo) * 128 + hi])

            # ---- S3: TT8 = x_TA^T (f-major) via banded fp8 matmul ----
            tt8 = pp.tile([128, DCH * N], F8, tag="tt8")
            for p in range(DCH):
                pb = psb.tile([128, N], FP, tag="big")
                qs = bands[p]
                q0 = qs[0][0]
                s0 = soff[(p, q0)]
                for h in range(2):
                    nc.tensor.matmul(
                        pb[:, h * 512:(h + 1) * 512],
                        ebig[:, s0 * 128:(s0 + 2) * 128].rearrange(
                            "q (k c) -> q k c", k=2),
                        xt8fv[:, q0:q0 + 2, h * 512:(h + 1) * 512],
                        start=True, stop=(len(qs) == 2), perf_mode=DR)
                    if len(qs) == 3:
                        q2 = qs[2][0]
                        s2 = soff[(p, q2)]
                        nc.tensor.matmul(
                            pb[:, h * 512:(h + 1) * 512],
                            ebig[:, s2 * 128:(s2 + 1) * 128],
                            xt8f[:, q2 * N + h * 512:q2 * N + (h + 1) * 512],
                            start=False, stop=True)
                copy_rr(tt8[:, p * N:(p + 1) * N], pb[:])
            tt8v = tt8[:].rearrange("q (p n) -> q p n", p=DCH)

            # ---- S5 (score_s -> SG) interleaved with S4 (AN build) ----
            sg8 = pp.tile([128, NCH * N], F8, tag="sg8")
            sg8v = sg8[:].rearrange("q (m n) -> q m n", m=NCH)
            anb = pp.tile([128, NCH * D], F8, tag="anb")
            anbv = anb[:].rearrange("q (m d) -> q m d", m=NCH)
            for i in range(NCH):
                pb = psb.tile([128, N], FP, tag="big")
                for h in range(2):
                    for a2 in range(3):  # DoubleRow over d-tile pairs
                        nc.tensor.matmul(
                            pb[:, h * 512:(h + 1) * 512],
                            tt8v[:, 2 * a2:2 * a2 + 2, i * 128:(i + 1) * 128],
                            tt8v[:, 2 * a2:2 * a2 + 2, h * 512:(h + 1) * 512],
                            start=(a2 == 0), stop=(a2 == 2), perf_mode=DR)
                nc.scalar.activation(sg8[:, i * N:(i + 1) * N], pb[:], AF.Sigmoid)

                # fp8 transpose must write psum with element step 2
                pa = pst.tile([128, 2 * D], F8, tag="tr")
                pav = pa[:].rearrange("q (c two) -> q two c", two=2)
                for p in range(DCH):
                    nc.tensor.transpose(pav[:, 0, p * 128:(p + 1) * 128],
                                        tt8[:, p * N + i * 128:p * N + (i + 1) * 128],
                                        ident8[:])
                # one strided copy: drop the step-2 padding and permute
                # f-major d -> t-major d' in the same instruction
                copy_rr(anb[:, i * D:(i + 1) * D].rearrange(
                            "q (t f) -> q f t", t=T, f=F).unsqueeze(3),
                        pa[:].rearrange("q (f t two) -> q f t two",
                                        f=F, t=T, two=2)[:, :, :, 0:1])

            # ---- S6: S_att rows + softmax; S7: transpose into SATB ----
            satb = pp.tile([128, NCH * N], F8, tag="satb")
            satbv = satb[:].rearrange("q (m n) -> q m n", m=NCH)
            for i in range(NCH):
                bsb = sp.tile([128, N], BF, tag="bsb", bufs=2)
                nc.sync.dma_start(bsb[:], bst_d[i * 128:(i + 1) * 128, :])
                pb = psb.tile([128, N], FP, tag="big")
                for h in range(2):
                    for a2 in range(4):  # DoubleRow over m-tile pairs
                        nc.tensor.matmul(
                            pb[:, h * 512:(h + 1) * 512],
                            vst8v[:, 2 * a2:2 * a2 + 2, i * 128:(i + 1) * 128],
                            sg8v[:, 2 * a2:2 * a2 + 2, h * 512:(h + 1) * 512],
                            start=(a2 == 0), stop=False, perf_mode=DR)
                    nc.tensor.matmul(
                        pb[:, h * 512:(h + 1) * 512],
                        identb[:], bsb[:, h * 512:(h + 1) * 512],
                        start=False, stop=True)
                sexp = sp.tile([128, N], FP, tag="sexp", bufs=2)
                ssum = sp.tile([128, 1], FP, tag="ssum", bufs=2)
                nc.scalar.activation(sexp[:], pb[:], AF.Exp, accum_out=ssum[:])
                sinv = sp.tile([128, 1], FP, tag="sinv", bufs=2)
                nc.vector.reciprocal(sinv[:], ssum[:])
                # x512 keeps softmax weights above the fp8e4m3 subnormal floor;
                # the Z0T drain divides it back out.
                sa = sp.tile([128, N], BF, tag="sa", bufs=2)
                nc.vector.tensor_scalar(sa[:], sexp[:], sinv[:], 512.0,
                                        op0=OP.mult, op1=OP.mult)
                for grp in range(2):
                    pq = pst.tile([128, 512], BF, tag="tr")
                    for k in range(4):
                        p = grp * 4 + k
                        nc.tensor.transpose(pq[:, k * 128:(k + 1) * 128],
                                            sa[:, p * 128:(p + 1) * 128],
                                            identb[:])
                    dst = satb[:].rearrange("q (p n) -> q p n", p=NCH)[
                        :, grp * 4:grp * 4 + 4, i * 128:(i + 1) * 128]
                    copy_rr(dst, pq[:].rearrange("q (p n) -> q p n", p=4))

            # 8*L^T as one (128, 8x1024) fp8 tile (m-tile blocks on cols)
            lt8 = pp.tile([128, NCH * N], F8, tag="lt8")
            for m in range(NCH):
                nc.sync.dma_start(lt8[:, m * N:(m + 1) * N],
                                  lt8_d[m * 128:(m + 1) * 128, :])
            lt8v = lt8[:].rearrange("q (m n) -> q m n", m=NCH)

            # ---- S8: Z0T = (S_att @ x_TA)^T directly (t-major) ----
            Z0T = []
            for p in range(DCH):
                pb = psb.tile([128, N], FP, tag="big")
                for h in range(2):
                    for a2 in range(4):  # DoubleRow over m-tile pairs
                        nc.tensor.matmul(
                            pb[:, h * 512:(h + 1) * 512],
                            anbv[:, 2 * a2:2 * a2 + 2, p * 128:(p + 1) * 128],
                            satbv[:, 2 * a2:2 * a2 + 2, h * 512:(h + 1) * 512],
                            start=(a2 == 0), stop=(a2 == 3), perf_mode=DR)
                t_ = pp.tile([128, N], BF, name=f"z0tT{p}", tag=f"T{p}")
                if p % 2 == 0:
                    nc.vector.tensor_scalar_mul(t_[:], pb[:], 1.0 / 512.0)
                else:
                    nc.scalar.activation(t_[:], pb[:], AF.Copy, scale=1.0 / 512.0)
                Z0T.append(t_)

            # ---- S9: Z0N = 64*transpose(Z0T), fp8 (m-blocks on cols) ----
            z0nb = pp.tile([128, NCH * D], F8, tag="z0nb")
            z0nbv = z0nb[:].rearrange("q (m d) -> q m d", m=NCH)
            for i in range(NCH):
                for grp in range(2):
                    pz = pst.tile([128, 384], BF, tag="tr")
                    for k in range(3):
                        p = grp * 3 + k
                        nc.tensor.transpose(pz[:, k * 128:(k + 1) * 128],
                                            Z0T[p][:, i * 128:(i + 1) * 128],
                                            identb[:])
                    scaled_rr(z0nb[:, i * D + grp * 384:i * D + (grp + 1) * 384],
                              pz[:], 64.0)

            # ---- S10: Z1T[d',n] = sum_m Z0[m,d'] L^T[m,n] = (L@Z0)^T ----
            Z1T = []
            for p in range(DCH):
                pb = psb.tile([128, N], FP, tag="big")
                for h in range(2):
                    for a2 in range(4):
                        nc.tensor.matmul(
                            pb[:, h * 512:(h + 1) * 512],
                            z0nbv[:, 2 * a2:2 * a2 + 2, p * 128:(p + 1) * 128],
                            lt8v[:, 2 * a2:2 * a2 + 2, h * 512:(h + 1) * 512],
                            start=(a2 == 0), stop=(a2 == 3), perf_mode=DR)
                t_ = pp.tile([128, N], BF, name=f"z1tV{p}", tag=f"V{p}")
                scaled_rr(t_[:], pb[:], 1.0 / 512.0)
                Z1T.append(t_)

            # ---- S11: Z1N = 64*transpose(Z1T), fp8 ----
            z1nb = pp.tile([128, NCH * D], F8, tag="z1nb")
            z1nbv = z1nb[:].rearrange("q (m d) -> q m d", m=NCH)
            for i in range(NCH):
                for grp in range(2):
                    pz = pst.tile([128, 384], BF, tag="tr")
                    for k in range(3):
                        p = grp * 3 + k
                        nc.tensor.transpose(pz[:, k * 128:(k + 1) * 128],
                                            Z1T[p][:, i * 128:(i + 1) * 128],
                                            identb[:])
                    scaled_rr(z1nb[:, i * D + grp * 384:i * D + (grp + 1) * 384],
                              pz[:], 64.0)

            # ---- S12: Z2T = 2*(L@Z1)^T - Z0T ----
            Z2T = []
            for p in range(DCH):
                pb = psb.tile([128, N], FP, tag="big")
                for h in range(2):
                    for a2 in range(4):
                        nc.tensor.matmul(
                            pb[:, h * 512:(h + 1) * 512],
                            z1nbv[:, 2 * a2:2 * a2 + 2, p * 128:(p + 1) * 128],
                            lt8v[:, 2 * a2:2 * a2 + 2, h * 512:(h + 1) * 512],
                            start=(a2 == 0), stop=(a2 == 3), perf_mode=DR)
                # psum holds 512*(L@Z1); Z2 = psum/256 - Z0
                zc = sp.tile([128, N], BF, tag="z2c", bufs=2)
                scaled_rr(zc[:], pb[:], 1.0 / 256.0)
                t_ = pp.tile([128, N], BF, name=f"z2tZ{p}", tag=f"Z2{p}")
                if p % 2 == 0:
                    nc.vector.tensor_tensor(t_[:], zc[:], Z0T[p][:], op=OP.subtract)
                else:
                    nc.gpsimd.tensor_tensor(t_[:], zc[:], Z0T[p][:], op=OP.subtract)
                Z2T.append(t_)

            # ---- S13: projection (Cheb k=0..2 + residual), bias, relu ----
            for q in range(QO):
                p = q // 2
                pb = psb.tile([128, N], FP, tag="big")
                rhs4 = (Z0T[p], Z1T[p], Z2T[p], XTT[p])
                for h in range(2):
                    for k in range(4):
                        nc.tensor.matmul(
                            pb[:, h * 512:(h + 1) * 512],
                            wpb[:, (4 * q + k) * 128:(4 * q + k + 1) * 128],
                            rhs4[k][:, h * 512:(h + 1) * 512],
                            start=(k == 0), stop=(k == 3))
                ob = sp.tile([128, N], BF, tag="outbuf", bufs=2)
                nc.scalar.activation(ob[:], pb[:], AF.Relu, bias=bias128[:])
                nc.sync.dma_start(out_d[q * 128:(q + 1) * 128, :], ob[:])

    nc.compile()
    _compiled["nc"] = nc
    return nc


def _host_prep(x, edge_index, edge_weight, Ve, be, Vs, bs, cheb_W, cheb_b, res_W, res_b):
    import ml_dtypes
    BF = ml_dtypes.bfloat16
    row = np.asarray(edge_index[0]).astype(np.int64)
    col = np.asarray(edge_index[1]).astype(np.int64)
    w = np.asarray(edge_weight, np.float64).copy()
    w[row == col] = 0.0
    deg = np.zeros(N, np.float64)
    np.add.at(deg, row, w)
    dis = np.where(deg > 0, 1.0 / np.sqrt(np.where(deg > 0, deg, 1.0)), 0.0)
    norm = -dis[row] * w * dis[col]
    L = np.zeros((N, N), np.float64)
    np.add.at(L, (col, row), norm)
    LT = np.ascontiguousarray(L.T.astype(np.float32))

    cheb_W = np.asarray(cheb_W, np.float32)
    res_W = np.asarray(res_W, np.float32)
    # wpb[p, (4q+k)*128 + c] = blk(q,k)[p, c]; out^T tile q rows (t,g) with
    # t = 2q + c//64, contracting t-major tile p=q//2 rows (t', f)
    wq = np.zeros((QO, 4, 128, 128), np.float32)
    Wlist = [cheb_W[0], cheb_W[1], cheb_W[2], res_W.T]  # each (F, G)
    for q in range(QO):
        off = 0 if q % 2 == 0 else 2
        for b_ in range(2):
            a = b_ + off
            for k in range(4):
                wq[q, k, 32 * a:32 * a + 32, 64 * b_:64 * b_ + 64] = Wlist[k]
    wpb = np.ascontiguousarray(
        wq.transpose(2, 0, 1, 3).reshape(128, QO * 4 * 128)).astype(BF)

    b64 = (np.asarray(cheb_b, np.float32) + np.asarray(res_b, np.float32))
    bias128 = np.concatenate([b64, b64]).reshape(128, 1).astype(np.float32)

    import ml_dtypes as mld
    bigi = np.zeros((128, 384), np.float32)
    bigi[np.arange(128), 128 + np.arange(128)] = 1.0
    return {
        "bigi": bigi.astype(mld.bfloat16),
        "identb": np.eye(128, dtype=np.float32).astype(BF),
        "ident8": np.eye(128, dtype=np.float32).astype(mld.float8_e4m3),
        "vetb": np.ascontiguousarray(np.asarray(Ve, np.float32).T).astype(BF),
        "be": np.ascontiguousarray(np.asarray(be, np.float32)[0]),
        "vst8": np.ascontiguousarray(
            np.asarray(Vs, np.float32).T).astype(mld.float8_e4m3),
        "bst": np.ascontiguousarray(np.asarray(bs, np.float32)[0]).astype(BF),
        "lt8": (8.0 * LT).astype(mld.float8_e4m3),
        "wpb": wpb,
        "bias128": bias128,
    }


TRACE = False
LAST = {}


def kernel(x, edge_index, edge_weight, Ve, be, Vs, bs, cheb_W, cheb_b, res_W, res_b):
    from concourse.bass_utils import run_bass_kernel_spmd
    import ml_dtypes
    BF = ml_dtypes.bfloat16
    F8H = ml_dtypes.float8_e4m3

    x = np.asarray(x, np.float32)
    shared = _host_prep(x, edge_index, edge_weight, Ve, be, Vs, bs,
                        cheb_W, cheb_b, res_W, res_b)
    nc = _build()
    in_maps = []
    for b in range(B):
        m = dict(shared)
        xb = x[b]                                   # (N, F, T)
        xnp = np.zeros((N, F, 32), np.float32)      # col 32f+t, zero padded
        xnp[:, :, :T] = xb
        m["xnp"] = np.ascontiguousarray(xnp.reshape(N, 1024)).astype(BF)
        xtf = xb.reshape(N, D).T                    # (768, N), d = f*24+t
        m["xt8f"] = np.ascontiguousarray(xtf).astype(F8H)
        m["xtt"] = np.ascontiguousarray(
            xb.transpose(2, 1, 0).reshape(D, N)).astype(BF)  # d' = t*32+f
        in_maps.append(m)
    res = run_bass_kernel_spmd(nc, in_maps, list(range(B)), trace=TRACE)
    LAST["res"] = res
    out = np.stack(
        [r["out"].astype(np.float32).reshape(T, G, N).transpose(2, 1, 0)
         for r in res.results], axis=0)
    return out


# revision 36
# speedup vs baseline: 3.2454x; 1.0962x over previous
"""STBlock (temporal attn -> spatial attn -> ChebConv + residual, relu) on 8 trn2 cores.

Sharding: data-parallel over batch B=8, one batch element per core.

v2 design notes (vs 509us baseline): the baseline burned ~93us of PE on 256
tiny 24-col transposes and ~300us of Vector/Scalar on per-instruction copy
overhead. This version:
  - uploads x from host in all three layouts it is consumed in (natural
    f-major, transposed f-major, transposed t-major), killing the stage-3
    transpose storm entirely;
  - keeps a t-major (d' = t*32+f) column order for every intermediate, so the
    final Cheb+residual projection is 12 plain 128-contract matmuls with
    block weights and zero permutes/transposes at the tail;
  - computes each Chebyshev propagation directly in transposed form
    (Z1^T = sum_m Z0[m,d'] * L^T[m,n]), halving transpose passes;
  - applies E_att via a banded 768x768 block-diagonal matmul (15 tile pairs)
    built on-device from eatt with quadrant-safe small copies;
  - folds the bs-add into the S_pre PSUM accumulation via an identity matmul,
    and skips softmax max-subtraction (scores are provably < ~5);
  - batches every PSUM->SBUF drain to >=384-col single instructions and
    round-robins them across Vector/GpSimd/Scalar.

Layouts (per core):
  d  = f*24+t (f-major), d' = t*32+f (t-major); out^T row = t*64+g.
  All partition offsets must be 32-aligned (BIR verifier quadrant rule), so
  f-blocks (24 rows/cols) are placed at 32-strides with zero padding.
  XNP[8]  (128n, 1024)  bf16   natural padded (col 32f+t), for score_t
  XT96P[8](128d+,1024n) bf16   x^T f-major padded (row 32j+u per 4-f group)
  XTT[6]  (128d',1024n) bf16   x^T t-major, residual rhs in projection
  TT96[8] (96d, 1024n)  bf16   x_TA^T compact f-major (E-mult out)
  AN[8]   (128n, 768d') bf16   x_TA natural t-major (transpose+permute of TT)
  SG[8]   (128n, 1024m) bf16   sigmoid(score_s)
  SATB    (128m, 8x1024n) bf16 S_att^T, m-tile blocks along free dim
  Z0T/Z1T/Z2T[6] (128d', 1024n) bf16; Z0N/Z1N[8] (128n, 768d') bf16
  out^T   (1536, 1024) bf16 -> host transposes back
"""
import numpy as np

B, N, F, T, G = 8, 1024, 32, 24, 64
D = F * T            # 768
NCH = N // 128       # 8 n-chunks
DCH = D // 128       # 6 d-tiles
QO = 12              # out^T tiles (1536 rows)

_compiled = {}


def _build():
    if "nc" in _compiled:
        return _compiled["nc"]
    import concourse.mybir as mybir
    import concourse.bacc as bacc
    from concourse import tile

    FP = mybir.dt.float32
    BF = mybir.dt.bfloat16
    F8 = mybir.dt.float8e4
    AF = mybir.ActivationFunctionType
    OP = mybir.AluOpType
    DR = mybir.MatmulPerfMode.DoubleRow

    nc = bacc.Bacc("TRN2", target_bir_lowering=False, debug=False)

    xnp_d = nc.dram_tensor("xnp", (N, 1024), BF, kind="ExternalInput").ap()
    xt8f_d = nc.dram_tensor("xt8f", (D, N), F8, kind="ExternalInput").ap()
    bigi_d = nc.dram_tensor("bigi", (128, 384), BF, kind="ExternalInput").ap()
    xtt_d = nc.dram_tensor("xtt", (D, N), BF, kind="ExternalInput").ap()
    identb_d = nc.dram_tensor("identb", (128, 128), BF, kind="ExternalInput").ap()
    ident8_d = nc.dram_tensor("ident8", (128, 128), F8, kind="ExternalInput").ap()
    vetb_d = nc.dram_tensor("vetb", (T, T), BF, kind="ExternalInput").ap()
    be_d = nc.dram_tensor("be", (T, T), FP, kind="ExternalInput").ap()
    vst_d = nc.dram_tensor("vst8", (N, N), F8, kind="ExternalInput").ap()
    bst_d = nc.dram_tensor("bst", (N, N), BF, kind="ExternalInput").ap()
    lt8_d = nc.dram_tensor("lt8", (N, N), F8, kind="ExternalInput").ap()
    wpb_d = nc.dram_tensor("wpb", (128, QO * 4 * 128), BF, kind="ExternalInput").ap()
    bias_d = nc.dram_tensor("bias128", (128, 1), FP, kind="ExternalInput").ap()
    out_d = nc.dram_tensor("out", (QO * 128, N), BF, kind="ExternalOutput").ap()

    with tile.TileContext(nc) as tc:
        with (
            tc.tile_pool(name="persist", bufs=1) as pp,
            tc.tile_pool(name="stream", bufs=1) as sp,
            tc.tile_pool(name="psb", bufs=2, space="PSUM") as psb,
            tc.tile_pool(name="pst", bufs=3, space="PSUM") as pst,
            tc.tile_pool(name="ps1", bufs=1, space="PSUM") as ps1,
        ):
            # round-robin for copy/cast work across DVE / Pool engines
            # (Act is kept for activations + a share of copies where idle)
            _rr = [0]
            PSUM_SPACE = tile.bass.MemorySpace.PSUM

            def copy_rr(dst, src, engines=None):
                if engines is None:
                    # GpSimd cannot touch PSUM
                    if src.space == PSUM_SPACE or dst.space == PSUM_SPACE:
                        engines = (nc.vector, nc.scalar)
                    else:
                        engines = (nc.vector, nc.gpsimd)
                e = engines[_rr[0] % len(engines)]
                _rr[0] += 1
                if e is nc.scalar:
                    nc.scalar.activation(dst, src, AF.Copy)
                else:
                    e.tensor_copy(dst, src)

            def scaled_rr(dst, src, scale):
                if _rr[0] % 2 == 0:
                    nc.vector.tensor_scalar_mul(dst, src, scale)
                else:
                    nc.scalar.activation(dst, src, AF.Copy, scale=scale)
                _rr[0] += 1

            # ---- constants / inputs ----
            identb = pp.tile([128, 128], BF, tag="identb")
            nc.sync.dma_start(identb[:], identb_d[:])
            ident8 = pp.tile([128, 128], F8, tag="ident8")
            nc.sync.dma_start(ident8[:], ident8_d[:])
            vetb = pp.tile([T, T], BF, tag="vetb")
            nc.sync.dma_start(vetb[:], vetb_d[:])
            be = pp.tile([T, T], FP, tag="be")
            nc.sync.dma_start(be[:], be_d[:])
            # preload Act function tables off the critical path
            warm = sp.tile([1, 1], FP, tag="warm")
            nc.scalar.activation(warm[:], identb[0:1, 0:1], AF.Sigmoid)
            nc.scalar.activation(warm[:], identb[0:1, 0:1], AF.Exp)
            nc.scalar.activation(warm[:], identb[0:1, 0:1], AF.Relu)
            wpb = pp.tile([128, QO * 4 * 128], BF, tag="wpb")
            nc.sync.dma_start(wpb[:], wpb_d[:])
            bias128 = pp.tile([128, 1], FP, tag="bias128")
            nc.sync.dma_start(bias128[:], bias_d[:])

            XNP = []
            for i in range(NCH):
                t_ = pp.tile([128, 1024], BF, name=f"xnpA{i}", tag=f"A{i}")
                nc.sync.dma_start(t_[:], xnp_d[i * 128:(i + 1) * 128, :])
                XNP.append(t_)
            # x^T f-major fp8, one tile: col block p = d-tile p (DR pairing)
            xt8f = pp.tile([128, DCH * N], F8, tag="xt8f")
            for p in range(DCH):
                nc.sync.dma_start(xt8f[:, p * N:(p + 1) * N],
                                  xt8f_d[p * 128:(p + 1) * 128, :])
            xt8fv = xt8f[:].rearrange("q (p n) -> q p n", p=DCH)
            bigi = pp.tile([128, 384], BF, tag="bigi")
            nc.sync.dma_start(bigi[:], bigi_d[:])
            XTT = []
            for p in range(DCH):
                t_ = pp.tile([128, N], BF, name=f"xttX{p}", tag=f"X{p}")
                nc.sync.dma_start(t_[:], xtt_d[p * 128:(p + 1) * 128, :])
                XTT.append(t_)
            # Vs^T as one (128, 8x1024) fp8 tile: col block m holds m-tile rows
            vst8 = pp.tile([128, NCH * N], F8, tag="vst8")
            for m in range(NCH):
                nc.sync.dma_start(vst8[:, m * N:(m + 1) * N],
                                  vst_d[m * 128:(m + 1) * 128, :])
            vst8v = vst8[:].rearrange("q (m n) -> q m n", m=NCH)

            # ---- S1: score_t = sum_{n,f} x[n,f,t] x[n,f,u] ----
            # XNP col blocks of 128 = 4 f's at 32-stride padding; the Gram of
            # each block has the per-f 24x24 diagonal blocks at 32-aligned
            # partition offsets. Garbage off-diagonal blocks are ignored.
            acc128 = pp.tile([128, 128], FP, tag="acc128")
            for g2 in range(8):
                pt = ps1.tile([128, 128], FP, tag="st")
                for i in range(NCH):
                    sl = XNP[i][:, g2 * 128:(g2 + 1) * 128]
                    nc.tensor.matmul(pt[:], sl, sl,
                                     start=(i == 0), stop=(i == NCH - 1))
                if g2 == 0:
                    nc.vector.tensor_copy(acc128[:], pt[:])
                else:
                    nc.vector.tensor_tensor(acc128[:], acc128[:], pt[:], op=OP.add)
            # TensorTensor needs equal base partitions for SBUF inputs, so
            # first move the three off-base diagonal blocks down to base 0.
            dg = []
            for j, eng in ((1, nc.vector), (2, nc.gpsimd), (3, nc.vector)):
                t_ = sp.tile([T, T], FP, name=f"dg{j}", tag=f"dg{j}")
                eng.tensor_copy(t_[:], acc128[32 * j:32 * j + 24,
                                              32 * j:32 * j + 24])
                dg.append(t_)
            sct_a = sp.tile([T, T], FP, tag="sct_a")
            nc.vector.tensor_tensor(sct_a[:], acc128[0:24, 0:24],
                                    dg[0][:], op=OP.add)
            sct_b = sp.tile([T, T], FP, tag="sct_b")
            nc.gpsimd.tensor_tensor(sct_b[:], dg[1][:], dg[2][:], op=OP.add)
            score_t = sp.tile([T, T], FP, tag="score_t")
            nc.vector.tensor_tensor(score_t[:], sct_a[:], sct_b[:], op=OP.add)

            # ---- S2: E_att = softmax(Ve @ sigmoid(score_t) + be) ----
            sigb = sp.tile([T, T], BF, tag="sigb")
            nc.scalar.activation(sigb[:], score_t[:], AF.Sigmoid)
            ps_e = ps1.tile([T, T], FP, tag="st")
            nc.tensor.matmul(ps_e[:], vetb[:], sigb[:], start=True, stop=True)
            epre = sp.tile([T, T], FP, tag="epre")
            nc.vector.tensor_tensor(epre[:], ps_e[:], be[:], op=OP.add)
            eexp = sp.tile([T, T], FP, tag="eexp")
            esum = sp.tile([T, 1], FP, tag="esum")
            nc.scalar.activation(eexp[:], epre[:], AF.Exp, accum_out=esum[:])
            einv = sp.tile([T, 1], FP, tag="einv")
            nc.vector.reciprocal(einv[:], esum[:])
            eatt = sp.tile([T, T], BF, tag="eatt")
            nc.vector.tensor_scalar_mul(eatt[:], eexp[:], einv[:])

            # EBIG: banded blocks of blockdiag(E_att x32), built on the PE
            # with shift-matrix (identity-slice) matmuls, then cast to fp8.
            bands = []
            for p in range(DCH):
                qs = []
                for q in (p - 1, p, p + 1):
                    if not 0 <= q < DCH:
                        continue
                    fs = [f for f in range(F)
                          if 24 * f < 128 * q + 128 and 24 * f + 24 > 128 * q
                          and 24 * f < 128 * p + 128 and 24 * f + 24 > 128 * p]
                    if fs:
                        qs.append((q, fs))
                bands.append(qs)
            soff = {}
            s = 0
            for p in range(DCH):
                for q, _ in bands[p]:
                    soff[(p, q)] = s
                    s += 1
            NB = s  # 14 blocks
            e4r = pp.tile([128, T], BF, tag="e4r")
            nc.gpsimd.memset(e4r[:], 0.0)
            nc.vector.tensor_copy(e4r[0:24, :], eatt[:])
            ebig = pp.tile([128, NB * 128], F8, tag="ebig")
            nc.gpsimd.memset(ebig[:], 0.0)
            for half in range(2):
                blo = half * 7
                bhi = min(NB, blo + 7)
                pe_b = psb.tile([128, N], FP, tag="big")
                ranges = {}
                for p in range(DCH):
                    for q, fs in bands[p]:
                        sb = soff[(p, q)]
                        if not blo <= sb < bhi:
                            continue
                        for f in fs:
                            dlt = 24 * f - 128 * q
                            c0 = 24 * f - 128 * p
                            t0, t1 = max(0, -c0), min(24, 128 - c0)
                            cc = (sb - blo) * 128 + c0 + t0
                            nc.tensor.matmul(
                                pe_b[:, cc:cc + (t1 - t0)],
                                bigi[:, 128 - dlt:256 - dlt],
                                e4r[:, t0:t1], start=True, stop=True)
                            lo, hi = ranges.get(sb, (10 ** 9, -1))
                            ranges[sb] = (min(lo, c0 + t0), max(hi, c0 + t1))
                for sb, (lo, hi) in sorted(ranges.items()):
                    copy_rr(ebig[:, sb * 128 + lo:sb * 128 + hi],
                            pe_b[:, (sb - blo) * 128 + lo:(sb - blo) * 128 + hi])

            # ---- S3: TT8 = x_TA^T (f-major) via banded fp8 matmul ----
            tt8 = pp.tile([128, DCH * N], F8, tag="tt8")
            for p in range(DCH):
                pb = psb.tile([128, N], FP, tag="big")
                qs = bands[p]
                q0 = qs[0][0]
                s0 = soff[(p, q0)]
                for h in range(2):
                    nc.tensor.matmul(
                        pb[:, h * 512:(h + 1) * 512],
                        ebig[:, s0 * 128:(s0 + 2) * 128].rearrange(
                            "q (k c) -> q k c", k=2),
                        xt8fv[:, q0:q0 + 2, h * 512:(h + 1) * 512],
                        start=True, stop=(len(qs) == 2), perf_mode=DR)
                    if len(qs) == 3:
                        q2 = qs[2][0]
                        s2 = soff[(p, q2)]
                        nc.tensor.matmul(
                            pb[:, h * 512:(h + 1) * 512],
                            ebig[:, s2 * 128:(s2 + 1) * 128],
                            xt8f[:, q2 * N + h * 512:q2 * N + (h + 1) * 512],
                            start=False, stop=True)
                copy_rr(tt8[:, p * N:(p + 1) * N], pb[:])
            tt8v = tt8[:].rearrange("q (p n) -> q p n", p=DCH)

            # ---- S5 (score_s -> SG) interleaved with S4 (AN build) ----
            sg8 = pp.tile([128, NCH * N], F8, tag="sg8")
            sg8v = sg8[:].rearrange("q (m n) -> q m n", m=NCH)
            # x_TA natural, t-major cols, fp8 (m-blocks along free dim)
            anb = pp.tile([128, NCH * D], F8, tag="anb")
            anbv = anb[:].rearrange("q (m d) -> q m d", m=NCH)
            for i in range(NCH):
                pb = psb.tile([128, N], FP, tag="big")
                for h in range(2):
                    for a2 in range(3):  # DoubleRow over d-tile pairs
                        nc.tensor.matmul(
                            pb[:, h * 512:(h + 1) * 512],
                            tt8v[:, 2 * a2:2 * a2 + 2, i * 128:(i + 1) * 128],
                            tt8v[:, 2 * a2:2 * a2 + 2, h * 512:(h + 1) * 512],
                            start=(a2 == 0), stop=(a2 == 2), perf_mode=DR)
                nc.scalar.activation(sg8[:, i * N:(i + 1) * N], pb[:], AF.Sigmoid)

                # fp8 transpose must write psum with element step 2
                pa = pst.tile([128, 2 * D], F8, tag="tr")
                pav = pa[:].rearrange("q (c two) -> q two c", two=2)
                for p in range(DCH):
                    nc.tensor.transpose(pav[:, 0, p * 128:(p + 1) * 128],
                                        tt8[:, p * N + i * 128:p * N + (i + 1) * 128],
                                        ident8[:])
                # permute f-major step-2 psum -> t-major fp8; contiguous
                # 32B writes per t; halves run on DVE and Act in parallel
                dstv = anb[:, i * D:(i + 1) * D].rearrange(
                    "q (t f) -> q t f", t=T, f=F)
                srcv = pa[:].rearrange("q (f t two) -> q t f two",
                                       f=F, t=T, two=2)
                nc.vector.tensor_copy(dstv[:, 0:12, :].unsqueeze(3),
                                      srcv[:, 0:12, :, 0:1])
                nc.scalar.activation(dstv[:, 12:24, :].unsqueeze(3),
                                     srcv[:, 12:24, :, 0:1], AF.Copy)

            # ---- S6: S_att rows + softmax; S7: transpose into SATB ----
            satb = pp.tile([128, NCH * N], F8, tag="satb")
            satbv = satb[:].rearrange("q (m n) -> q m n", m=NCH)
            for i in range(NCH):
                bsb = sp.tile([128, N], BF, tag="bsb", bufs=2)
                nc.sync.dma_start(bsb[:], bst_d[i * 128:(i + 1) * 128, :])
                pb = psb.tile([128, N], FP, tag="big")
                for h in range(2):
                    for a2 in range(4):  # DoubleRow over m-tile pairs
                        nc.tensor.matmul(
                            pb[:, h * 512:(h + 1) * 512],
                            vst8v[:, 2 * a2:2 * a2 + 2, i * 128:(i + 1) * 128],
                            sg8v[:, 2 * a2:2 * a2 + 2, h * 512:(h + 1) * 512],
                            start=(a2 == 0), stop=False, perf_mode=DR)
                    nc.tensor.matmul(
                        pb[:, h * 512:(h + 1) * 512],
                        identb[:], bsb[:, h * 512:(h + 1) * 512],
                        start=False, stop=True)
                sexp = sp.tile([128, N], FP, tag="sexp", bufs=2)
                ssum = sp.tile([128, 1], FP, tag="ssum", bufs=2)
                nc.scalar.activation(sexp[:], pb[:], AF.Exp, accum_out=ssum[:])
                sinv = sp.tile([128, 1], FP, tag="sinv", bufs=2)
                nc.vector.reciprocal(sinv[:], ssum[:])
                # x512 keeps softmax weights above the fp8e4m3 subnormal floor;
                # the Z0T drain divides it back out.
                sa = sp.tile([128, N], BF, tag="sa", bufs=2)
                nc.vector.tensor_scalar(sa[:], sexp[:], sinv[:], 512.0,
                                        op0=OP.mult, op1=OP.mult)
                for grp in range(2):
                    pq = pst.tile([128, 512], BF, tag="tr")
                    for k in range(4):
                        p = grp * 4 + k
                        nc.tensor.transpose(pq[:, k * 128:(k + 1) * 128],
                                            sa[:, p * 128:(p + 1) * 128],
                                            identb[:])
                    dst = satb[:].rearrange("q (p n) -> q p n", p=NCH)[
                        :, grp * 4:grp * 4 + 4, i * 128:(i + 1) * 128]
                    copy_rr(dst, pq[:].rearrange("q (p n) -> q p n", p=4))

            # 8*L^T as one (128, 8x1024) fp8 tile (m-tile blocks on cols)
            lt8 = pp.tile([128, NCH * N], F8, tag="lt8")
            for m in range(NCH):
                nc.sync.dma_start(lt8[:, m * N:(m + 1) * N],
                                  lt8_d[m * 128:(m + 1) * 128, :])
            lt8v = lt8[:].rearrange("q (m n) -> q m n", m=NCH)

            # ---- S8: Z0T = (S_att @ x_TA)^T directly (t-major) ----
            Z0T = []
            for p in range(DCH):
                pb = psb.tile([128, N], FP, tag="big")
                for h in range(2):
                    for a2 in range(4):  # DoubleRow over m-tile pairs
                        nc.tensor.matmul(
                            pb[:, h * 512:(h + 1) * 512],
                            anbv[:, 2 * a2:2 * a2 + 2, p * 128:(p + 1) * 128],
                            satbv[:, 2 * a2:2 * a2 + 2, h * 512:(h + 1) * 512],
                            start=(a2 == 0), stop=(a2 == 3), perf_mode=DR)
                t_ = pp.tile([128, N], BF, name=f"z0tT{p}", tag=f"T{p}")
                if p % 2 == 0:
                    nc.vector.tensor_scalar_mul(t_[:], pb[:], 1.0 / 512.0)
                else:
                    nc.scalar.activation(t_[:], pb[:], AF.Copy, scale=1.0 / 512.0)
                Z0T.append(t_)

            # ---- S9: Z0N = 64*transpose(Z0T), fp8 (m-blocks on cols) ----
            z0nb = pp.tile([128, NCH * D], F8, tag="z0nb")
            z0nbv = z0nb[:].rearrange("q (m d) -> q m d", m=NCH)
            for i in range(NCH):
                for grp in range(2):
                    pz = pst.tile([128, 384], BF, tag="tr")
                    for k in range(3):
                        p = grp * 3 + k
                        nc.tensor.transpose(pz[:, k * 128:(k + 1) * 128],
                                            Z0T[p][:, i * 128:(i + 1) * 128],
                                            identb[:])
                    scaled_rr(z0nb[:, i * D + grp * 384:i * D + (grp + 1) * 384],
                              pz[:], 64.0)

            # ---- S10: Z1T[d',n] = sum_m Z0[m,d'] L^T[m,n] = (L@Z0)^T ----
            Z1T = []
            for p in range(DCH):
                pb = psb.tile([128, N], FP, tag="big")
                for h in range(2):
                    for a2 in range(4):
                        nc.tensor.matmul(
                            pb[:, h * 512:(h + 1) * 512],
                            z0nbv[:, 2 * a2:2 * a2 + 2, p * 128:(p + 1) * 128],
                            lt8v[:, 2 * a2:2 * a2 + 2, h * 512:(h + 1) * 512],
                            start=(a2 == 0), stop=(a2 == 3), perf_mode=DR)
                t_ = pp.tile([128, N], BF, name=f"z1tV{p}", tag=f"V{p}")
                scaled_rr(t_[:], pb[:], 1.0 / 512.0)
                Z1T.append(t_)

            # ---- S11: Z1N = 64*transpose(Z1T), fp8 ----
            z1nb = pp.tile([128, NCH * D], F8, tag="z1nb")
            z1nbv = z1nb[:].rearrange("q (m d) -> q m d", m=NCH)
            for i in range(NCH):
                for grp in range(2):
                    pz = pst.tile([128, 384], BF, tag="tr")
                    for k in range(3):
                        p = grp * 3 + k
                        nc.tensor.transpose(pz[:, k * 128:(k + 1) * 128],
                                            Z1T[p][:, i * 128:(i + 1) * 128],
                                            identb[:])
                    scaled_rr(z1nb[:, i * D + grp * 384:i * D + (grp + 1) * 384],
                              pz[:], 64.0)

            # ---- S12: Z2T = 2*(L@Z1)^T - Z0T ----
            Z2T = []
            for p in range(DCH):
                pb = psb.tile([128, N], FP, tag="big")
                for h in range(2):
                    for a2 in range(4):
                        nc.tensor.matmul(
                            pb[:, h * 512:(h + 1) * 512],
                            z1nbv[:, 2 * a2:2 * a2 + 2, p * 128:(p + 1) * 128],
                            lt8v[:, 2 * a2:2 * a2 + 2, h * 512:(h + 1) * 512],
                            start=(a2 == 0), stop=(a2 == 3), perf_mode=DR)
                # psum holds 512*(L@Z1); Z2 = psum/256 - Z0
                zc = sp.tile([128, N], BF, tag="z2c", bufs=2)
                scaled_rr(zc[:], pb[:], 1.0 / 256.0)
                t_ = pp.tile([128, N], BF, name=f"z2tZ{p}", tag=f"Z2{p}")
                if p % 2 == 0:
                    nc.vector.tensor_tensor(t_[:], zc[:], Z0T[p][:], op=OP.subtract)
                else:
                    nc.gpsimd.tensor_tensor(t_[:], zc[:], Z0T[p][:], op=OP.subtract)
                Z2T.append(t_)

            # ---- S13: projection (Cheb k=0..2 + residual), bias, relu ----
            for q in range(QO):
                p = q // 2
                pb = psb.tile([128, N], FP, tag="big")
                rhs4 = (Z0T[p], Z1T[p], Z2T[p], XTT[p])
                for h in range(2):
                    for k in range(4):
                        nc.tensor.matmul(
                            pb[:, h * 512:(h + 1) * 512],
                            wpb[:, (4 * q + k) * 128:(4 * q + k + 1) * 128],
                            rhs4[k][:, h * 512:(h + 1) * 512],
                            start=(k == 0), stop=(k == 3))
                ob = sp.tile([128, N], BF, tag="outbuf", bufs=2)
                nc.scalar.activation(ob[:], pb[:], AF.Relu, bias=bias128[:])
                nc.sync.dma_start(out_d[q * 128:(q + 1) * 128, :], ob[:])

    nc.compile()
    _compiled["nc"] = nc
    return nc


def _host_prep(x, edge_index, edge_weight, Ve, be, Vs, bs, cheb_W, cheb_b, res_W, res_b):
    import ml_dtypes
    BF = ml_dtypes.bfloat16
    row = np.asarray(edge_index[0]).astype(np.int64)
    col = np.asarray(edge_index[1]).astype(np.int64)
    w = np.asarray(edge_weight, np.float64).copy()
    w[row == col] = 0.0
    deg = np.zeros(N, np.float64)
    np.add.at(deg, row, w)
    dis = np.where(deg > 0, 1.0 / np.sqrt(np.where(deg > 0, deg, 1.0)), 0.0)
    norm = -dis[row] * w * dis[col]
    L = np.zeros((N, N), np.float64)
    np.add.at(L, (col, row), norm)
    LT = np.ascontiguousarray(L.T.astype(np.float32))

    cheb_W = np.asarray(cheb_W, np.float32)
    res_W = np.asarray(res_W, np.float32)
    # wpb[p, (4q+k)*128 + c] = blk(q,k)[p, c]; out^T tile q rows (t,g) with
    # t = 2q + c//64, contracting t-major tile p=q//2 rows (t', f)
    wq = np.zeros((QO, 4, 128, 128), np.float32)
    Wlist = [cheb_W[0], cheb_W[1], cheb_W[2], res_W.T]  # each (F, G)
    for q in range(QO):
        off = 0 if q % 2 == 0 else 2
        for b_ in range(2):
            a = b_ + off
            for k in range(4):
                wq[q, k, 32 * a:32 * a + 32, 64 * b_:64 * b_ + 64] = Wlist[k]
    wpb = np.ascontiguousarray(
        wq.transpose(2, 0, 1, 3).reshape(128, QO * 4 * 128)).astype(BF)

    b64 = (np.asarray(cheb_b, np.float32) + np.asarray(res_b, np.float32))
    bias128 = np.concatenate([b64, b64]).reshape(128, 1).astype(np.float32)

    import ml_dtypes as mld
    bigi = np.zeros((128, 384), np.float32)
    bigi[np.arange(128), 128 + np.arange(128)] = 1.0
    return {
        "bigi": bigi.astype(mld.bfloat16),
        "identb": np.eye(128, dtype=np.float32).astype(BF),
        "ident8": np.eye(128, dtype=np.float32).astype(mld.float8_e4m3),
        "vetb": np.ascontiguousarray(np.asarray(Ve, np.float32).T).astype(BF),
        "be": np.ascontiguousarray(np.asarray(be, np.float32)[0]),
        "vst8": np.ascontiguousarray(
            np.asarray(Vs, np.float32).T).astype(mld.float8_e4m3),
        "bst": np.ascontiguousarray(np.asarray(bs, np.float32)[0]).astype(BF),
        "lt8": (8.0 * LT).astype(mld.float8_e4m3),
        "wpb": wpb,
        "bias128": bias128,
    }


TRACE = False
LAST = {}


def kernel(x, edge_index, edge_weight, Ve, be, Vs, bs, cheb_W, cheb_b, res_W, res_b):
    from concourse.bass_utils import run_bass_kernel_spmd
    import ml_dtypes
    BF = ml_dtypes.bfloat16
    F8H = ml_dtypes.float8_e4m3

    x = np.asarray(x, np.float32)
    shared = _host_prep(x, edge_index, edge_weight, Ve, be, Vs, bs,
                        cheb_W, cheb_b, res_W, res_b)
    nc = _build()
    in_maps = []
    for b in range(B):
        m = dict(shared)
        xb = x[b]                                   # (N, F, T)
        xnp = np.zeros((N, F, 32), np.float32)      # col 32f+t, zero padded
        xnp[:, :, :T] = xb
        m["xnp"] = np.ascontiguousarray(xnp.reshape(N, 1024)).astype(BF)
        xtf = xb.reshape(N, D).T                    # (768, N), d = f*24+t
        m["xt8f"] = np.ascontiguousarray(xtf).astype(F8H)
        m["xtt"] = np.ascontiguousarray(
            xb.transpose(2, 1, 0).reshape(D, N)).astype(BF)  # d' = t*32+f
        in_maps.append(m)
    res = run_bass_kernel_spmd(nc, in_maps, list(range(B)), trace=TRACE)
    LAST["res"] = res
    out = np.stack(
        [r["out"].astype(np.float32).reshape(T, G, N).transpose(2, 1, 0)
         for r in res.results], axis=0)
    return out


# revision 38
# speedup vs baseline: 3.3415x; 1.0296x over previous
"""STBlock (temporal attn -> spatial attn -> ChebConv + residual, relu) on 8 trn2 cores.

Sharding: data-parallel over batch B=8, one batch element per core.

v2 design notes (vs 509us baseline): the baseline burned ~93us of PE on 256
tiny 24-col transposes and ~300us of Vector/Scalar on per-instruction copy
overhead. This version:
  - uploads x from host in all three layouts it is consumed in (natural
    f-major, transposed f-major, transposed t-major), killing the stage-3
    transpose storm entirely;
  - keeps a t-major (d' = t*32+f) column order for every intermediate, so the
    final Cheb+residual projection is 12 plain 128-contract matmuls with
    block weights and zero permutes/transposes at the tail;
  - computes each Chebyshev propagation directly in transposed form
    (Z1^T = sum_m Z0[m,d'] * L^T[m,n]), halving transpose passes;
  - applies E_att via a banded 768x768 block-diagonal matmul (15 tile pairs)
    built on-device from eatt with quadrant-safe small copies;
  - folds the bs-add into the S_pre PSUM accumulation via an identity matmul,
    and skips softmax max-subtraction (scores are provably < ~5);
  - batches every PSUM->SBUF drain to >=384-col single instructions and
    round-robins them across Vector/GpSimd/Scalar.

Layouts (per core):
  d  = f*24+t (f-major), d' = t*32+f (t-major); out^T row = t*64+g.
  All partition offsets must be 32-aligned (BIR verifier quadrant rule), so
  f-blocks (24 rows/cols) are placed at 32-strides with zero padding.
  XNP[8]  (128n, 1024)  bf16   natural padded (col 32f+t), for score_t
  XT96P[8](128d+,1024n) bf16   x^T f-major padded (row 32j+u per 4-f group)
  XTT[6]  (128d',1024n) bf16   x^T t-major, residual rhs in projection
  TT96[8] (96d, 1024n)  bf16   x_TA^T compact f-major (E-mult out)
  AN[8]   (128n, 768d') bf16   x_TA natural t-major (transpose+permute of TT)
  SG[8]   (128n, 1024m) bf16   sigmoid(score_s)
  SATB    (128m, 8x1024n) bf16 S_att^T, m-tile blocks along free dim
  Z0T/Z1T/Z2T[6] (128d', 1024n) bf16; Z0N/Z1N[8] (128n, 768d') bf16
  out^T   (1536, 1024) bf16 -> host transposes back
"""
import numpy as np

B, N, F, T, G = 8, 1024, 32, 24, 64
D = F * T            # 768
NCH = N // 128       # 8 n-chunks
DCH = D // 128       # 6 d-tiles
QO = 12              # out^T tiles (1536 rows)

_compiled = {}


def _build():
    if "nc" in _compiled:
        return _compiled["nc"]
    import concourse.mybir as mybir
    import concourse.bacc as bacc
    from concourse import tile

    FP = mybir.dt.float32
    BF = mybir.dt.bfloat16
    F8 = mybir.dt.float8e4
    AF = mybir.ActivationFunctionType
    OP = mybir.AluOpType
    DR = mybir.MatmulPerfMode.DoubleRow

    nc = bacc.Bacc("TRN2", target_bir_lowering=False, debug=False)

    xnp_d = nc.dram_tensor("xnp", (N, 1024), F8, kind="ExternalInput").ap()
    xt8f_d = nc.dram_tensor("xt8f", (D, N), F8, kind="ExternalInput").ap()
    bigi_d = nc.dram_tensor("bigi", (128, 384), BF, kind="ExternalInput").ap()
    xtt_d = nc.dram_tensor("xtt", (D, N), BF, kind="ExternalInput").ap()
    identb_d = nc.dram_tensor("identb", (128, 128), BF, kind="ExternalInput").ap()
    ident8_d = nc.dram_tensor("ident8", (128, 128), F8, kind="ExternalInput").ap()
    vetb_d = nc.dram_tensor("vetb", (T, T), BF, kind="ExternalInput").ap()
    be_d = nc.dram_tensor("be", (T, T), FP, kind="ExternalInput").ap()
    vst_d = nc.dram_tensor("vst8", (N, N), F8, kind="ExternalInput").ap()
    bst_d = nc.dram_tensor("bst", (N, N), BF, kind="ExternalInput").ap()
    lt8_d = nc.dram_tensor("lt8", (N, N), F8, kind="ExternalInput").ap()
    wpb_d = nc.dram_tensor("wpb", (128, QO * 4 * 128), BF, kind="ExternalInput").ap()
    bias_d = nc.dram_tensor("bias128", (128, 1), FP, kind="ExternalInput").ap()
    out_d = nc.dram_tensor("out", (QO * 128, N), BF, kind="ExternalOutput").ap()

    with tile.TileContext(nc) as tc:
        with (
            tc.tile_pool(name="persist", bufs=1) as pp,
            tc.tile_pool(name="stream", bufs=1) as sp,
            tc.tile_pool(name="psb", bufs=2, space="PSUM") as psb,
            tc.tile_pool(name="pst", bufs=3, space="PSUM") as pst,
            tc.tile_pool(name="ps1", bufs=1, space="PSUM") as ps1,
        ):
            # round-robin for copy/cast work across DVE / Pool engines
            # (Act is kept for activations + a share of copies where idle)
            _rr = [0]
            PSUM_SPACE = tile.bass.MemorySpace.PSUM

            def copy_rr(dst, src, engines=None):
                if engines is None:
                    # GpSimd cannot touch PSUM
                    if src.space == PSUM_SPACE or dst.space == PSUM_SPACE:
                        engines = (nc.vector, nc.scalar)
                    else:
                        engines = (nc.vector, nc.gpsimd)
                e = engines[_rr[0] % len(engines)]
                _rr[0] += 1
                if e is nc.scalar:
                    nc.scalar.activation(dst, src, AF.Copy)
                else:
                    e.tensor_copy(dst, src)

            def scaled_rr(dst, src, scale):
                if _rr[0] % 2 == 0:
                    nc.vector.tensor_scalar_mul(dst, src, scale)
                else:
                    nc.scalar.activation(dst, src, AF.Copy, scale=scale)
                _rr[0] += 1

            # ---- constants / inputs ----
            identb = pp.tile([128, 128], BF, tag="identb")
            nc.sync.dma_start(identb[:], identb_d[:])
            ident8 = pp.tile([128, 128], F8, tag="ident8")
            nc.sync.dma_start(ident8[:], ident8_d[:])
            vetb = pp.tile([T, T], BF, tag="vetb")
            nc.sync.dma_start(vetb[:], vetb_d[:])
            be = pp.tile([T, T], FP, tag="be")
            nc.sync.dma_start(be[:], be_d[:])
            # preload Act function tables off the critical path
            warm = sp.tile([1, 1], FP, tag="warm")
            nc.scalar.activation(warm[:], identb[0:1, 0:1], AF.Sigmoid)
            nc.scalar.activation(warm[:], identb[0:1, 0:1], AF.Exp)
            nc.scalar.activation(warm[:], identb[0:1, 0:1], AF.Relu)

            XNP = []
            for i in range(NCH):
                t_ = pp.tile([128, 1024], F8, name=f"xnpA{i}", tag=f"A{i}")
                nc.sync.dma_start(t_[:], xnp_d[i * 128:(i + 1) * 128, :])
                XNP.append(t_)
            # x^T f-major fp8, one tile: col block p = d-tile p (DR pairing)
            xt8f = pp.tile([128, DCH * N], F8, tag="xt8f")
            for p in range(DCH):
                nc.sync.dma_start(xt8f[:, p * N:(p + 1) * N],
                                  xt8f_d[p * 128:(p + 1) * 128, :])
            xt8fv = xt8f[:].rearrange("q (p n) -> q p n", p=DCH)
            bigi = pp.tile([128, 384], BF, tag="bigi")
            nc.sync.dma_start(bigi[:], bigi_d[:])
            # Vs^T as one (128, 8x1024) fp8 tile: col block m holds m-tile rows
            vst8 = pp.tile([128, NCH * N], F8, tag="vst8")
            for m in range(NCH):
                nc.sync.dma_start(vst8[:, m * N:(m + 1) * N],
                                  vst_d[m * 128:(m + 1) * 128, :])
            vst8v = vst8[:].rearrange("q (m n) -> q m n", m=NCH)

            # ---- S1: score_t = sum_{n,f} x[n,f,t] x[n,f,u] ----
            # XNP col blocks of 128 = 4 f's at 32-stride padding; the Gram of
            # each block has the per-f 24x24 diagonal blocks at 32-aligned
            # partition offsets. Garbage off-diagonal blocks are ignored.
            acc128 = pp.tile([128, 128], FP, tag="acc128")
            for g2 in range(8):
                pt = ps1.tile([128, 128], FP, tag="st")
                for i in range(NCH):
                    sl = XNP[i][:, g2 * 128:(g2 + 1) * 128]
                    nc.tensor.matmul(pt[:], sl, sl,
                                     start=(i == 0), stop=(i == NCH - 1))
                if g2 == 0:
                    nc.vector.tensor_copy(acc128[:], pt[:])
                else:
                    nc.vector.tensor_tensor(acc128[:], acc128[:], pt[:], op=OP.add)
            # TensorTensor needs equal base partitions for SBUF inputs, so
            # first move the three off-base diagonal blocks down to base 0.
            dg = []
            for j, eng in ((1, nc.vector), (2, nc.gpsimd), (3, nc.vector)):
                t_ = sp.tile([T, T], FP, name=f"dg{j}", tag=f"dg{j}")
                eng.tensor_copy(t_[:], acc128[32 * j:32 * j + 24,
                                              32 * j:32 * j + 24])
                dg.append(t_)
            sct_a = sp.tile([T, T], FP, tag="sct_a")
            nc.vector.tensor_tensor(sct_a[:], acc128[0:24, 0:24],
                                    dg[0][:], op=OP.add)
            sct_b = sp.tile([T, T], FP, tag="sct_b")
            nc.gpsimd.tensor_tensor(sct_b[:], dg[1][:], dg[2][:], op=OP.add)
            score_t = sp.tile([T, T], FP, tag="score_t")
            nc.vector.tensor_tensor(score_t[:], sct_a[:], sct_b[:], op=OP.add)

            # ---- S2: E_att = softmax(Ve @ sigmoid(score_t) + be) ----
            sigb = sp.tile([T, T], BF, tag="sigb")
            nc.scalar.activation(sigb[:], score_t[:], AF.Sigmoid)
            ps_e = ps1.tile([T, T], FP, tag="st")
            nc.tensor.matmul(ps_e[:], vetb[:], sigb[:], start=True, stop=True)
            epre = sp.tile([T, T], FP, tag="epre")
            nc.vector.tensor_tensor(epre[:], ps_e[:], be[:], op=OP.add)
            eexp = sp.tile([T, T], FP, tag="eexp")
            esum = sp.tile([T, 1], FP, tag="esum")
            nc.scalar.activation(eexp[:], epre[:], AF.Exp, accum_out=esum[:])
            einv = sp.tile([T, 1], FP, tag="einv")
            nc.vector.reciprocal(einv[:], esum[:])
            eatt = sp.tile([T, T], BF, tag="eatt")
            nc.vector.tensor_scalar_mul(eatt[:], eexp[:], einv[:])

            # EBIG: banded blocks of blockdiag(E_att x32), built on the PE
            # with shift-matrix (identity-slice) matmuls, then cast to fp8.
            bands = []
            for p in range(DCH):
                qs = []
                for q in (p - 1, p, p + 1):
                    if not 0 <= q < DCH:
                        continue
                    fs = [f for f in range(F)
                          if 24 * f < 128 * q + 128 and 24 * f + 24 > 128 * q
                          and 24 * f < 128 * p + 128 and 24 * f + 24 > 128 * p]
                    if fs:
                        qs.append((q, fs))
                bands.append(qs)
            soff = {}
            s = 0
            for p in range(DCH):
                for q, _ in bands[p]:
                    soff[(p, q)] = s
                    s += 1
            NB = s  # 14 blocks
            e4r = pp.tile([128, T], BF, tag="e4r")
            nc.gpsimd.memset(e4r[:], 0.0)
            nc.vector.tensor_copy(e4r[0:24, :], eatt[:])
            ebig = pp.tile([128, NB * 128], F8, tag="ebig")
            nc.gpsimd.memset(ebig[:], 0.0)
            for half in range(2):
                blo = half * 7
                bhi = min(NB, blo + 7)
                pe_b = psb.tile([128, N], FP, tag="big")
                ranges = {}
                for p in range(DCH):
                    for q, fs in bands[p]:
                        sb = soff[(p, q)]
                        if not blo <= sb < bhi:
                            continue
                        for f in fs:
                            dlt = 24 * f - 128 * q
                            c0 = 24 * f - 128 * p
                            t0, t1 = max(0, -c0), min(24, 128 - c0)
                            cc = (sb - blo) * 128 + c0 + t0
                            nc.tensor.matmul(
                                pe_b[:, cc:cc + (t1 - t0)],
                                bigi[:, 128 - dlt:256 - dlt],
                                e4r[:, t0:t1], start=True, stop=True)
                            lo, hi = ranges.get(sb, (10 ** 9, -1))
                            ranges[sb] = (min(lo, c0 + t0), max(hi, c0 + t1))
                for sb, (lo, hi) in sorted(ranges.items()):
                    copy_rr(ebig[:, sb * 128 + lo:sb * 128 + hi],
                            pe_b[:, (sb - blo) * 128 + lo:(sb - blo) * 128 + hi])

            # ---- S3: TT8 = x_TA^T (f-major) via banded fp8 matmul ----
            tt8 = pp.tile([128, DCH * N], F8, tag="tt8")
            for p in range(DCH):
                pb = psb.tile([128, N], FP, tag="big")
                qs = bands[p]
                q0 = qs[0][0]
                s0 = soff[(p, q0)]
                for h in range(2):
                    nc.tensor.matmul(
                        pb[:, h * 512:(h + 1) * 512],
                        ebig[:, s0 * 128:(s0 + 2) * 128].rearrange(
                            "q (k c) -> q k c", k=2),
                        xt8fv[:, q0:q0 + 2, h * 512:(h + 1) * 512],
                        start=True, stop=(len(qs) == 2), perf_mode=DR)
                    if len(qs) == 3:
                        q2 = qs[2][0]
                        s2 = soff[(p, q2)]
                        nc.tensor.matmul(
                            pb[:, h * 512:(h + 1) * 512],
                            ebig[:, s2 * 128:(s2 + 1) * 128],
                            xt8f[:, q2 * N + h * 512:q2 * N + (h + 1) * 512],
                            start=False, stop=True)
                copy_rr(tt8[:, p * N:(p + 1) * N], pb[:])
            tt8v = tt8[:].rearrange("q (p n) -> q p n", p=DCH)

            # ---- S5 (score_s -> SG) interleaved with S4 (AN build) ----
            sg8 = pp.tile([128, NCH * N], F8, tag="sg8")
            sg8v = sg8[:].rearrange("q (m n) -> q m n", m=NCH)
            # x_TA natural, t-major cols, fp8 (m-blocks along free dim)
            anb = pp.tile([128, NCH * D], F8, tag="anb")
            anbv = anb[:].rearrange("q (m d) -> q m d", m=NCH)
            for i in range(NCH):
                pb = psb.tile([128, N], FP, tag="big")
                for h in range(2):
                    for a2 in range(3):  # DoubleRow over d-tile pairs
                        nc.tensor.matmul(
                            pb[:, h * 512:(h + 1) * 512],
                            tt8v[:, 2 * a2:2 * a2 + 2, i * 128:(i + 1) * 128],
                            tt8v[:, 2 * a2:2 * a2 + 2, h * 512:(h + 1) * 512],
                            start=(a2 == 0), stop=(a2 == 2), perf_mode=DR)
                nc.scalar.activation(sg8[:, i * N:(i + 1) * N], pb[:], AF.Sigmoid)

                # fp8 transpose must write psum with element step 2
                pa = pst.tile([128, 2 * D], F8, tag="tr")
                pav = pa[:].rearrange("q (c two) -> q two c", two=2)
                for p in range(DCH):
                    nc.tensor.transpose(pav[:, 0, p * 128:(p + 1) * 128],
                                        tt8[:, p * N + i * 128:p * N + (i + 1) * 128],
                                        ident8[:])
                # permute f-major step-2 psum -> t-major fp8; contiguous
                # 32B writes per t; halves run on DVE and Act in parallel
                dstv = anb[:, i * D:(i + 1) * D].rearrange(
                    "q (t f) -> q t f", t=T, f=F)
                srcv = pa[:].rearrange("q (f t two) -> q t f two",
                                       f=F, t=T, two=2)
                nc.vector.tensor_copy(dstv[:, 0:12, :].unsqueeze(3),
                                      srcv[:, 0:12, :, 0:1])
                nc.scalar.activation(dstv[:, 12:24, :].unsqueeze(3),
                                     srcv[:, 12:24, :, 0:1], AF.Copy)

            # ---- S6: S_att rows + softmax; S7: transpose into SATB ----
            satb = pp.tile([128, NCH * N], F8, tag="satb")
            satbv = satb[:].rearrange("q (m n) -> q m n", m=NCH)
            for i in range(NCH):
                bsb = sp.tile([128, N], BF, tag="bsb", bufs=2)
                nc.sync.dma_start(bsb[:], bst_d[i * 128:(i + 1) * 128, :])
                pb = psb.tile([128, N], FP, tag="big")
                for h in range(2):
                    for a2 in range(4):  # DoubleRow over m-tile pairs
                        nc.tensor.matmul(
                            pb[:, h * 512:(h + 1) * 512],
                            vst8v[:, 2 * a2:2 * a2 + 2, i * 128:(i + 1) * 128],
                            sg8v[:, 2 * a2:2 * a2 + 2, h * 512:(h + 1) * 512],
                            start=(a2 == 0), stop=False, perf_mode=DR)
                    nc.tensor.matmul(
                        pb[:, h * 512:(h + 1) * 512],
                        identb[:], bsb[:, h * 512:(h + 1) * 512],
                        start=False, stop=True)
                sexp = sp.tile([128, N], FP, tag="sexp", bufs=2)
                ssum = sp.tile([128, 1], FP, tag="ssum", bufs=2)
                nc.scalar.activation(sexp[:], pb[:], AF.Exp, accum_out=ssum[:])
                sinv = sp.tile([128, 1], FP, tag="sinv", bufs=2)
                nc.vector.reciprocal(sinv[:], ssum[:])
                # x512 keeps softmax weights above the fp8e4m3 subnormal floor;
                # the Z0T drain divides it back out.
                sa = sp.tile([128, N], BF, tag="sa", bufs=2)
                nc.gpsimd.tensor_scalar(sa[:], sexp[:], sinv[:], 512.0,
                                        op0=OP.mult, op1=OP.mult)
                for grp in range(2):
                    pq = pst.tile([128, 512], BF, tag="tr")
                    for k in range(4):
                        p = grp * 4 + k
                        nc.tensor.transpose(pq[:, k * 128:(k + 1) * 128],
                                            sa[:, p * 128:(p + 1) * 128],
                                            identb[:])
                    dst = satb[:].rearrange("q (p n) -> q p n", p=NCH)[
                        :, grp * 4:grp * 4 + 4, i * 128:(i + 1) * 128]
                    copy_rr(dst, pq[:].rearrange("q (p n) -> q p n", p=4))

            # late DMAs: not needed before S8+, keep head bandwidth clear
            wpb = pp.tile([128, QO * 4 * 128], BF, tag="wpb")
            nc.sync.dma_start(wpb[:], wpb_d[:])
            bias128 = pp.tile([128, 1], FP, tag="bias128")
            nc.sync.dma_start(bias128[:], bias_d[:])
            XTT = []
            for p in range(DCH):
                t_ = pp.tile([128, N], BF, name=f"xttX{p}", tag=f"X{p}")
                nc.sync.dma_start(t_[:], xtt_d[p * 128:(p + 1) * 128, :])
                XTT.append(t_)
            # 8*L^T as one (128, 8x1024) fp8 tile (m-tile blocks on cols)
            lt8 = pp.tile([128, NCH * N], F8, tag="lt8")
            for m in range(NCH):
                nc.sync.dma_start(lt8[:, m * N:(m + 1) * N],
                                  lt8_d[m * 128:(m + 1) * 128, :])
            lt8v = lt8[:].rearrange("q (m n) -> q m n", m=NCH)

            # ---- S8: Z0T = (S_att @ x_TA)^T directly (t-major) ----
            Z0T = []
            for p in range(DCH):
                pb = psb.tile([128, N], FP, tag="big")
                for h in range(2):
                    for a2 in range(4):  # DoubleRow over m-tile pairs
                        nc.tensor.matmul(
                            pb[:, h * 512:(h + 1) * 512],
                            anbv[:, 2 * a2:2 * a2 + 2, p * 128:(p + 1) * 128],
                            satbv[:, 2 * a2:2 * a2 + 2, h * 512:(h + 1) * 512],
                            start=(a2 == 0), stop=(a2 == 3), perf_mode=DR)
                t_ = pp.tile([128, N], BF, name=f"z0tT{p}", tag=f"T{p}")
                if p % 2 == 0:
                    nc.vector.tensor_scalar_mul(t_[:], pb[:], 1.0 / 512.0)
                else:
                    nc.scalar.activation(t_[:], pb[:], AF.Copy, scale=1.0 / 512.0)
                Z0T.append(t_)

            # ---- S9: Z0N = 64*transpose(Z0T), fp8 (m-blocks on cols) ----
            z0nb = pp.tile([128, NCH * D], F8, tag="z0nb")
            z0nbv = z0nb[:].rearrange("q (m d) -> q m d", m=NCH)
            for i in range(NCH):
                for grp in range(2):
                    pz = pst.tile([128, 384], BF, tag="tr")
                    for k in range(3):
                        p = grp * 3 + k
                        nc.tensor.transpose(pz[:, k * 128:(k + 1) * 128],
                                            Z0T[p][:, i * 128:(i + 1) * 128],
                                            identb[:])
                    scaled_rr(z0nb[:, i * D + grp * 384:i * D + (grp + 1) * 384],
                              pz[:], 64.0)

            # ---- S10: Z1T[d',n] = sum_m Z0[m,d'] L^T[m,n] = (L@Z0)^T ----
            Z1T = []
            for p in range(DCH):
                pb = psb.tile([128, N], FP, tag="big")
                for h in range(2):
                    for a2 in range(4):
                        nc.tensor.matmul(
                            pb[:, h * 512:(h + 1) * 512],
                            z0nbv[:, 2 * a2:2 * a2 + 2, p * 128:(p + 1) * 128],
                            lt8v[:, 2 * a2:2 * a2 + 2, h * 512:(h + 1) * 512],
                            start=(a2 == 0), stop=(a2 == 3), perf_mode=DR)
                t_ = pp.tile([128, N], BF, name=f"z1tV{p}", tag=f"V{p}")
                scaled_rr(t_[:], pb[:], 1.0 / 512.0)
                Z1T.append(t_)

            # ---- S11: Z1N = 64*transpose(Z1T), fp8 ----
            z1nb = pp.tile([128, NCH * D], F8, tag="z1nb")
            z1nbv = z1nb[:].rearrange("q (m d) -> q m d", m=NCH)
            for i in range(NCH):
                for grp in range(2):
                    pz = pst.tile([128, 384], BF, tag="tr")
                    for k in range(3):
                        p = grp * 3 + k
                        nc.tensor.transpose(pz[:, k * 128:(k + 1) * 128],
                                            Z1T[p][:, i * 128:(i + 1) * 128],
                                            identb[:])
                    scaled_rr(z1nb[:, i * D + grp * 384:i * D + (grp + 1) * 384],
                              pz[:], 64.0)

            # ---- S12: Z2T = 2*(L@Z1)^T - Z0T ----
            Z2T = []
            for p in range(DCH):
                pb = psb.tile([128, N], FP, tag="big")
                for h in range(2):
                    for a2 in range(4):
                        nc.tensor.matmul(
                            pb[:, h * 512:(h + 1) * 512],
                            z1nbv[:, 2 * a2:2 * a2 + 2, p * 128:(p + 1) * 128],
                            lt8v[:, 2 * a2:2 * a2 + 2, h * 512:(h + 1) * 512],
                            start=(a2 == 0), stop=(a2 == 3), perf_mode=DR)
                # psum holds 512*(L@Z1); Z2 = psum/256 - Z0
                zc = sp.tile([128, N], BF, tag="z2c", bufs=2)
                scaled_rr(zc[:], pb[:], 1.0 / 256.0)
                t_ = pp.tile([128, N], BF, name=f"z2tZ{p}", tag=f"Z2{p}")
                if p % 2 == 0:
                    nc.vector.tensor_tensor(t_[:], zc[:], Z0T[p][:], op=OP.subtract)
                else:
                    nc.gpsimd.tensor_tensor(t_[:], zc[:], Z0T[p][:], op=OP.subtract)
                Z2T.append(t_)

            # ---- S13: projection (Cheb k=0..2 + residual), bias, relu ----
            for q in range(QO):
                p = q // 2
                pb = psb.tile([128, N], FP, tag="big")
                rhs4 = (Z0T[p], Z1T[p], Z2T[p], XTT[p])
                for h in range(2):
                    for k in range(4):
                        nc.tensor.matmul(
                            pb[:, h * 512:(h + 1) * 512],
                            wpb[:, (4 * q + k) * 128:(4 * q + k + 1) * 128],
                            rhs4[k][:, h * 512:(h + 1) * 512],
                            start=(k == 0), stop=(k == 3))
                ob = sp.tile([128, N], BF, tag="outbuf", bufs=2)
                if q % 2 == 0:
                    nc.scalar.activation(ob[:], pb[:], AF.Relu, bias=bias128[:])
                else:
                    nc.vector.tensor_scalar(ob[:], pb[:], bias128[:], 0.0,
                                            op0=OP.add, op1=OP.max)
                nc.sync.dma_start(out_d[q * 128:(q + 1) * 128, :], ob[:])

    nc.compile()
    _compiled["nc"] = nc
    return nc


def _host_prep(x, edge_index, edge_weight, Ve, be, Vs, bs, cheb_W, cheb_b, res_W, res_b):
    import ml_dtypes
    BF = ml_dtypes.bfloat16
    row = np.asarray(edge_index[0]).astype(np.int64)
    col = np.asarray(edge_index[1]).astype(np.int64)
    w = np.asarray(edge_weight, np.float64).copy()
    w[row == col] = 0.0
    deg = np.zeros(N, np.float64)
    np.add.at(deg, row, w)
    dis = np.where(deg > 0, 1.0 / np.sqrt(np.where(deg > 0, deg, 1.0)), 0.0)
    norm = -dis[row] * w * dis[col]
    L = np.zeros((N, N), np.float64)
    np.add.at(L, (col, row), norm)
    LT = np.ascontiguousarray(L.T.astype(np.float32))

    cheb_W = np.asarray(cheb_W, np.float32)
    res_W = np.asarray(res_W, np.float32)
    # wpb[p, (4q+k)*128 + c] = blk(q,k)[p, c]; out^T tile q rows (t,g) with
    # t = 2q + c//64, contracting t-major tile p=q//2 rows (t', f)
    wq = np.zeros((QO, 4, 128, 128), np.float32)
    Wlist = [cheb_W[0], cheb_W[1], cheb_W[2], res_W.T]  # each (F, G)
    for q in range(QO):
        off = 0 if q % 2 == 0 else 2
        for b_ in range(2):
            a = b_ + off
            for k in range(4):
                wq[q, k, 32 * a:32 * a + 32, 64 * b_:64 * b_ + 64] = Wlist[k]
    wpb = np.ascontiguousarray(
        wq.transpose(2, 0, 1, 3).reshape(128, QO * 4 * 128)).astype(BF)

    b64 = (np.asarray(cheb_b, np.float32) + np.asarray(res_b, np.float32))
    bias128 = np.concatenate([b64, b64]).reshape(128, 1).astype(np.float32)

    import ml_dtypes as mld
    bigi = np.zeros((128, 384), np.float32)
    bigi[np.arange(128), 128 + np.arange(128)] = 1.0
    return {
        "bigi": bigi.astype(mld.bfloat16),
        "identb": np.eye(128, dtype=np.float32).astype(BF),
        "ident8": np.eye(128, dtype=np.float32).astype(mld.float8_e4m3),
        "vetb": np.ascontiguousarray(np.asarray(Ve, np.float32).T).astype(BF),
        "be": np.ascontiguousarray(np.asarray(be, np.float32)[0]),
        "vst8": np.ascontiguousarray(
            np.asarray(Vs, np.float32).T).astype(mld.float8_e4m3),
        "bst": np.ascontiguousarray(np.asarray(bs, np.float32)[0]).astype(BF),
        "lt8": (8.0 * LT).astype(mld.float8_e4m3),
        "wpb": wpb,
        "bias128": bias128,
    }


TRACE = False
LAST = {}


def kernel(x, edge_index, edge_weight, Ve, be, Vs, bs, cheb_W, cheb_b, res_W, res_b):
    from concourse.bass_utils import run_bass_kernel_spmd
    import ml_dtypes
    BF = ml_dtypes.bfloat16
    F8H = ml_dtypes.float8_e4m3

    x = np.asarray(x, np.float32)
    shared = _host_prep(x, edge_index, edge_weight, Ve, be, Vs, bs,
                        cheb_W, cheb_b, res_W, res_b)
    nc = _build()
    in_maps = []
    for b in range(B):
        m = dict(shared)
        xb = x[b]                                   # (N, F, T)
        xnp = np.zeros((N, F, 32), np.float32)      # col 32f+t, zero padded
        xnp[:, :, :T] = xb
        m["xnp"] = np.ascontiguousarray(xnp.reshape(N, 1024)).astype(F8H)
        xtf = xb.reshape(N, D).T                    # (768, N), d = f*24+t
        m["xt8f"] = np.ascontiguousarray(xtf).astype(F8H)
        m["xtt"] = np.ascontiguousarray(
            xb.transpose(2, 1, 0).reshape(D, N)).astype(BF)  # d' = t*32+f
        in_maps.append(m)
    res = run_bass_kernel_spmd(nc, in_maps, list(range(B)), trace=TRACE)
    LAST["res"] = res
    out = np.stack(
        [r["out"].astype(np.float32).reshape(T, G, N).transpose(2, 1, 0)
         for r in res.results], axis=0)
    return out
